# revision 1
# baseline (speedup 1.0000x reference)
"""Distributed Trainium2 Bass kernel for nn_Attention_62766652063769 (v2).

Reference computation (B=4, T=2048, C=1024, H=16, HD=64):
    qkv = x @ W_qkv^T ; split into q, k, v heads
    q, k <- RoPE(q), RoPE(k)   (interleaved-pair rotation)
    attn = softmax(q k^T / sqrt(HD))   (mask is all-ones -> no masking)
    out  = (attn @ v) @ W_proj^T

Sharding: 8 cores; core c owns batch b = c//2 and query-token half c%2
(1024 q tokens).  K/V for the full 2048-token batch are computed
redundantly by both cores of a pair - zero inter-core communication.

v2 design (vs v1 baseline at ~605us):
  - The scalar (ACT) engine runs ONLY Exp the whole kernel (one table
    load).  The old per-pair softmax epilogue used Ln/Exp -> ACT table
    thrash (~2.7us per switch).  The denominator reciprocal moved to the
    DVE (RECIPROCAL_APPROX_FAST custom op).
  - Q/K/V projections of pair p+1 are emitted as "filler" grants inside
    pair p's attention loop, sharing one 2-slot PSUM ring with the score
    tiles.  This keeps the PE dense during the ACT-bound attention inner
    loop (no HAM re-throttle) and removes the serial QK prologue.
  - V is computed in 2-pair groups (N=256 matmuls, xt chunk stationary
    reused across the group) instead of per-pair N=128 matmuls that were
    LDWEIGHTS-bound.
  - Rope grants copy PSUM->SBUF bf16 first, then do the cos/sin muls in
    bf16 SBUF (DVE 2x mode) - shorter PSUM residency.
  - xt loads in 4 chunks (own-query-half columns first) so the first Q
    matmul starts ~6us in instead of ~21us.
  - W_proj loads into SBUF space freed by xt after the attention loop.

Layouts per core (SBUF, bf16 storage / fp32 PSUM):
    QT  [d=128(2 heads), pair, tq=1024]
    KT  [d=128, pair, tk=2048]
    V   [tk-tile 128, tt, head, 65]  (64 value dims + ones column)
    ST  [tk=128-tile, tq]  scores transposed, exp -> PT bf16
    OT  [65, tq] accumulated over tk tiles; row 64 = softmax denominator
    att = OT[0:64] * recip(denominator) broadcast (DRAM bounce)
    out = att^T-chunks @ W_proj^T-chunks
"""

import os
import re
import sys
import types

if "/opt/trn_rl_repo" not in sys.path:
    sys.path.insert(0, "/opt/trn_rl_repo")

import ml_dtypes
import numpy as np

import bass_rust
import concourse.bass as bass
import concourse.mybir as mybir
from concourse import bass_utils
from concourse.tile import TileContext, ScopedClock

# ---------------------------------------------------------------------------
# Environment patches (same as v1)
# ---------------------------------------------------------------------------

def _patched_drain_and_barrier(self, tick_clock, wait_clock):
    """The walrus build in this container encodes at most one sync-wait per
    instruction; Tile's tail drain carries one wait per live semaphore.
    Emit single-wait NOPs on SP instead, then an unguarded drain."""
    gc = tick_clock.global_clock
    ticks = [int(x) for x in re.findall(r"\d+", repr(gc))]
    for i, t in enumerate(ticks):
        if t <= 0:
            continue
        l = [0] * len(ticks)
        l[i] = t
        nop = self.nc.sync.nop(nofuse=True)
        wait_clock.add_sem_waits(nop.ins, ScopedClock({None: bass_rust.VectorClock(l)}))
    self.nc.sync.drain()
    self.nc.all_engine_barrier()
    assert self.sems is not None
    popped = self.nc._tile_sem_poison_stack.pop()
    assert popped is self._sem_poison
    self.nc.clear_and_free_semaphores(list(self.sems.allocated().values()))
    self.nc.all_engine_barrier()


TileContext._drain_and_barrier = _patched_drain_and_barrier


def _split_multi_waits(nc):
    """Move extra sync-waits onto single-wait NOPs inserted just before the
    owning instruction on the same (in-order) engine."""
    for func in nc.m.functions:
        for bb in func.blocks:
            insts = bb.instructions
            if not any(
                i.sync_info is not None
                and i.sync_info.on_wait
                and len(i.sync_info.on_wait) > 1
                for i in insts
            ):
                continue
            new = []
            for inst in insts:
                si = inst.sync_info
                if si is not None and si.on_wait and len(si.on_wait) > 1:
                    waits = list(si.on_wait)
                    for w in waits[:-1]:
                        nop = mybir.InstNoOp(
                            name=nc.get_next_instruction_name(),
                            engine=inst.engine,
                            bass_nofuse=True,
                            sync_info=mybir.SyncInfo(on_wait=[w], on_update=[]),
                        )
                        nc.register_instruction(nop)
                        new.append(nop)
                    inst.sync_info = mybir.SyncInfo(
                        on_wait=[waits[-1]], on_update=list(si.on_update)
                    )
                new.append(inst)
            bb.instructions = new


def _install_ntff_hook():
    """Recreate antenv.axon_hooks (absent in this image) so
    run_bass_kernel_spmd(trace=True) can profile through libaxon_pjrt."""
    if "antenv.axon_hooks" in sys.modules:
        return
    import contextlib
    import ctypes

    mod = types.ModuleType("antenv.axon_hooks")
    _state = {"hook": None}

    def set_axon_ntff_profile_hook(hook):
        _state["hook"] = hook

    def get_axon_ntff_profile_hook():
        return _state["hook"]

    def _ntff_profile_via_ctypes(so_path):
        lib = ctypes.CDLL(so_path)
        if not hasattr(lib, "axon_start_nrt_profile"):
            return None
        lib.axon_start_nrt_profile.argtypes = [
            ctypes.POINTER(ctypes.c_int64),
            ctypes.c_size_t,
        ]
        lib.axon_start_nrt_profile.restype = ctypes.c_int64
        lib.axon_stop_nrt_profile.argtypes = [ctypes.c_char_p]
        lib.axon_stop_nrt_profile.restype = ctypes.c_int64

        @contextlib.contextmanager
        def _hook(output_dir, device_ids):
            import jax

            jax.devices()
            if device_ids:
                ids = (ctypes.c_int64 * len(device_ids))(*device_ids)
                rc = lib.axon_start_nrt_profile(ids, len(device_ids))
            else:
                rc = lib.axon_start_nrt_profile(None, 0)
            if rc != 0:
                raise RuntimeError(f"axon_start_nrt_profile rc={rc}")
            try:
                yield
            finally:
                n = lib.axon_stop_nrt_profile(str(output_dir).encode())
                if n < 0:
                    raise RuntimeError(f"axon_stop_nrt_profile rc={n}")
                print(f"profile: {n} file(s) in {output_dir}", file=sys.stderr)

        return _hook

    mod.set_axon_ntff_profile_hook = set_axon_ntff_profile_hook
    mod.get_axon_ntff_profile_hook = get_axon_ntff_profile_hook
    try:
        set_axon_ntff_profile_hook(
            _ntff_profile_via_ctypes("/opt/axon/libaxon_pjrt.so")
        )
    except Exception:
        pass
    sys.modules["antenv.axon_hooks"] = mod
    try:
        import antenv

        antenv.axon_hooks = mod
    except ImportError:
        pass


_install_ntff_hook()

# ---------------------------------------------------------------------------
# Problem constants
# ---------------------------------------------------------------------------

B, T, C = 4, 2048, 1024
H, HD = 16, 64
NCORES = 8
TQ = T // 2          # q tokens per core
NPAIR = H // 2       # head pairs (=8); pair p holds heads 2p, 2p+1
NVG = NPAIR // 2     # V groups of 2 pairs (4 heads, 256 v dims)
KT_TILES = T // 128  # 16
SCALE = 1.0 / np.sqrt(HD)

F32 = mybir.dt.float32
BF16 = mybir.dt.bfloat16

CC = C // 128  # 8 contraction chunks


# ---------------------------------------------------------------------------
# Device program
# ---------------------------------------------------------------------------

def _build_nc():
    nc = bass.Bass(trn_type="TRN2", target_bir_lowering=False, debug=False)

    xt = nc.declare_dram_parameter("xt", [C, T], BF16, isOutput=False)
    wqt = nc.declare_dram_parameter("wqt", [NPAIR, 128, CC, 128], BF16,
                                    isOutput=False)
    wkt = nc.declare_dram_parameter("wkt", [NPAIR, 128, CC, 128], BF16,
                                    isOutput=False)
    wvg = nc.declare_dram_parameter("wvg", [NVG, 128, CC, 256], BF16,
                                    isOutput=False)
    wpt = nc.declare_dram_parameter("wpt", [C, C], BF16, isOutput=False)
    cosk = nc.declare_dram_parameter("cosk", [128, T], BF16, isOutput=False)
    sink = nc.declare_dram_parameter("sink", [128, T], BF16, isOutput=False)
    out_ext = nc.declare_dram_parameter("out", [TQ, C], F32, isOutput=True)

    rs_dram = nc.dram_tensor("rs_scratch", [NPAIR, 2, TQ], BF16)

    with TileContext(nc) as tc:
        with tc.tile_pool(name="persist", bufs=1) as persist:
            qt_sb = persist.tile([128, NPAIR, TQ], BF16, tag="qt")
            att_sb = persist.tile([128, NPAIR, TQ], BF16, tag="att")
            kt_sb = persist.tile([128, NPAIR, T], BF16, tag="kt")
            v_sb = persist.tile([128, KT_TILES, H, 65], BF16, tag="v")
            ck = persist.tile([128, T], BF16, tag="ck")
            sk = persist.tile([128, T], BF16, tag="sk")

            with tc.tile_pool(name="xtpool", bufs=1) as xtpool, \
                 tc.tile_pool(name="pw", bufs=1) as pw:
                xt_sb = xtpool.tile([128, CC, T], BF16, tag="xt")
                xt_r = xt.rearrange("(cc p) t -> p cc t", p=128)
                # DMA queue is serial: critical-path bytes first (weights
                # for pair 0 are loaded first inside _attention; the
                # partner-half xt chunks are emitted there too, after the
                # startup grants, so rope-swap DMAs aren't stuck behind
                # them).
                nc.vector.memset(v_sb[:, :, :, 64:65], 1.0)

                wp_sb = pw.tile([128, CC, C], BF16)

                _attention(nc, tc, xt_sb, qt_sb, kt_sb, v_sb, att_sb,
                           ck, sk, wqt, wkt, wvg, rs_dram, wp_sb, wpt,
                           xt_r, cosk, sink)

                _phase_proj(nc, tc, wp_sb, att_sb, out_ext)

    _split_multi_waits(nc)
    return nc


def _attention(nc, tc, xt_sb, qt_sb, kt_sb, v_sb, att_sb, ck, sk,
               wqt, wkt, wvg, rs_dram, wp_sb, wpt, xt_r, cosk, sink):
    with tc.tile_pool(name="ring", bufs=2, space="PSUM") as ring, \
         tc.tile_pool(name="otps", bufs=1, space="PSUM") as otps, \
         tc.tile_pool(name="wts", bufs=2) as wts, \
         tc.tile_pool(name="qkx", bufs=2) as qkx, \
         tc.tile_pool(name="ptp", bufs=4) as ptp, \
         tc.tile_pool(name="eps", bufs=1) as eps:

        wq_tiles = {}
        wk_tiles = {}
        wv_tiles = {}

        def load_wq(p):
            t = wts.tile([128, CC, 128], BF16, tag="wq")
            nc.sync.dma_start(out=t, in_=wqt[p])
            wq_tiles[p] = t

        def load_wk(p):
            t = wts.tile([128, CC, 128], BF16, tag="wk")
            nc.sync.dma_start(out=t, in_=wkt[p])
            wk_tiles[p] = t

        def load_wv(g):
            t = wts.tile([128, CC, 256], BF16, tag="wv")
            nc.sync.dma_start(out=t, in_=wvg[g])
            wv_tiles[g] = t

        def qk_grant(dst_sb, pair, wtile, col0, swap_eng=None):
            """One 512-token-column projection + rope grant -> dst_sb."""
            if swap_eng is None:
                swap_eng = nc.sync
            ps = ring.tile([128, TQ], F32, tag="ring", name=f"ps_{pair}_{col0}")
            for cc in range(CC):
                nc.tensor.matmul(
                    ps[:, 0:512],
                    lhsT=wtile[:, cc, :],
                    rhs=xt_sb[:, cc, col0:col0 + 512],
                    start=(cc == 0),
                    stop=(cc == CC - 1),
                )
            xb = qkx.tile([128, 512], BF16, tag="xb", name=f"xb_{pair}_{col0}")
            nc.vector.tensor_copy(xb, ps[:, 0:512])
            u = qkx.tile([128, 512], BF16, tag="u", name=f"u_{pair}_{col0}")
            v = qkx.tile([128, 512], BF16, tag="v", name=f"v_{pair}_{col0}")
            vs = qkx.tile([128, 512], BF16, tag="vs", name=f"vs_{pair}_{col0}")
            nc.vector.tensor_mul(u, xb, ck[:, col0:col0 + 512])
            nc.vector.tensor_mul(v, xb, sk[:, col0:col0 + 512])
            for blk in range(4):
                r = blk * 32
                s = (blk ^ 1) * 32
                swap_eng.dma_start(out=vs[r:r + 32, :], in_=v[s:s + 32, :])
            nc.gpsimd.tensor_add(dst_sb[:, pair, col0:col0 + 512], u, vs)

        def v_grant(g, tt):
            """V columns for group g (4 heads), token tile tt."""
            psv = ring.tile([128, 256], F32, tag="ring", name=f"psv_{g}_{tt}")
            for cc in range(CC):
                nc.tensor.matmul(
                    psv,
                    lhsT=xt_sb[:, cc, tt * 128:(tt + 1) * 128],
                    rhs=wv_tiles[g][:, cc, :],
                    start=(cc == 0),
                    stop=(cc == CC - 1),
                )
            nc.vector.tensor_copy(
                v_sb[:, tt, 4 * g:4 * g + 4, 0:64],
                psv.rearrange("p (h d) -> p h d", h=4),
            )

        # ------------------------------------------------------- startup
        # Minimal prologue: just enough for S(p0, kt0..3).  Everything
        # else (K quarters 1-3, all of V group 0) streams in as pair-0
        # fillers so the ACT exp pipeline starts ~40us earlier.
        load_wq(0)
        nc.sync.dma_start(out=xt_sb[:, 0:4, 0:TQ], in_=xt_r[:, 0:4, 0:TQ])
        nc.sync.dma_start(out=xt_sb[:, 4:8, 0:TQ], in_=xt_r[:, 4:8, 0:TQ])
        nc.scalar.dma_start(out=ck[:, 0:TQ], in_=cosk[:, 0:TQ])
        load_wk(0)
        load_wv(0)
        nc.scalar.dma_start(out=sk[:, 0:TQ], in_=sink[:, 0:TQ])
        qk_grant(qt_sb, 0, wq_tiles[0], 0, swap_eng=nc.scalar)
        qk_grant(qt_sb, 0, wq_tiles[0], 512, swap_eng=nc.scalar)
        qk_grant(kt_sb, 0, wk_tiles[0], 0, swap_eng=nc.scalar)
        nc.sync.dma_start(out=ck[:, TQ:T], in_=cosk[:, TQ:T])
        nc.sync.dma_start(out=sk[:, TQ:T], in_=sink[:, TQ:T])
        nc.sync.dma_start(out=xt_sb[:, 0:4, TQ:T], in_=xt_r[:, 0:4, TQ:T])
        nc.sync.dma_start(out=xt_sb[:, 4:8, TQ:T], in_=xt_r[:, 4:8, TQ:T])

        # ------------------------------------------------- attention loop
        for p in range(NPAIR):
            if p < 7:
                load_wq(p + 1)
                load_wk(p + 1)
            if p < 6 and p % 2 == 0:
                load_wv(p // 2 + 1)
            if p == 6:
                # preload W_proj so the proj phase starts immediately
                wp_r = wpt.rearrange("(cc p) e -> p cc e", p=128)
                nc.sync.dma_start(out=wp_sb[:, 0:4, :], in_=wp_r[:, 0:4, :])
                nc.sync.dma_start(out=wp_sb[:, 4:8, :], in_=wp_r[:, 4:8, :])

            def mk_v(g, tt):
                return lambda: v_grant(g, tt)

            def mk_k(q, n):
                return lambda: qk_grant(kt_sb, q, wk_tiles[q], n * 512)

            def mk_q(q, n):
                return lambda: qk_grant(qt_sb, q, wq_tiles[q], n * 512)

            if p == 0:
                # Prefill: emitted before the first S matmuls so the PE
                # has ready work while the Q/K rope chains complete.
                # Constraint: V0(tt) must be emitted at iteration <= tt
                # (OT(kt=tt) runs at iteration tt+1); K0 quarter n before
                # iteration 4n.  Pair-1 prefetch rides the back half.
                for f in [mk_v(0, 0), mk_v(0, 1), mk_v(0, 2), mk_k(0, 1),
                          mk_v(0, 3), mk_v(0, 4)]:
                    f()
                fill_at = {
                    0: [mk_v(0, 5), mk_v(0, 6)],
                    1: [mk_k(0, 2)],
                    2: [mk_v(0, 7), mk_v(0, 8)],
                    3: [mk_k(0, 3)],
                    4: [mk_v(0, 9), mk_v(0, 10)],
                    5: [mk_v(0, 11)],
                    6: [mk_v(0, 12)],
                    7: [mk_v(0, 13), mk_q(1, 0)],
                    8: [mk_v(0, 14), mk_q(1, 1)],
                    9: [mk_v(0, 15), mk_k(1, 0)],
                    10: [mk_k(1, 1)],
                    11: [mk_k(1, 2)],
                    12: [mk_k(1, 3)],
                    13: [mk_v(1, 0), mk_v(1, 1)],
                    14: [mk_v(1, 2), mk_v(1, 3)],
                    15: [mk_v(1, 4), mk_v(1, 5)],
                }
            else:
                fillers = []
                if p == 1:
                    fillers += [mk_v(1, tt) for tt in range(6, 16)]
                elif p < 6:
                    g = p // 2 + 1
                    for tt in range((p % 2) * 8, (p % 2) * 8 + 8):
                        fillers.append(mk_v(g, tt))
                if p < 7:
                    for n in range(4):
                        fillers.append(mk_k(p + 1, n))
                    for n in range(2):
                        fillers.append(mk_q(p + 1, n))
                # spread fillers across the 16 kt iterations
                fill_at = {}
                for i, f in enumerate(fillers):
                    fill_at.setdefault((i * 16) // max(len(fillers), 1),
                                       []).append(f)

            psAB = otps.tile([128, 2, TQ], F32, tag="ot", name=f"psAB_{p}")

            def ot_mm(kt, ptA, ptB):
                for nch in range(2):
                    nc.tensor.matmul(
                        psAB[0:65, 0, nch * 512:(nch + 1) * 512],
                        lhsT=v_sb[:, kt, 2 * p, :],
                        rhs=ptA[:, nch * 512:(nch + 1) * 512],
                        start=(kt == 0),
                        stop=(kt == KT_TILES - 1),
                    )
                    nc.tensor.matmul(
                        psAB[0:65, 1, nch * 512:(nch + 1) * 512],
                        lhsT=v_sb[:, kt, 2 * p + 1, :],
                        rhs=ptB[:, nch * 512:(nch + 1) * 512],
                        start=(kt == 0),
                        stop=(kt == KT_TILES - 1),
                    )

            # Software-pipelined: the PE is in-order, so OT(kt) must not
            # be enqueued before S(kt+1) — it would stall the whole PE
            # queue on exp(kt).  Defer OT by one kt.
            pend = None
            for kt in range(KT_TILES):
                stA = ring.tile([128, TQ], F32, tag="ring", name=f"stA_{p}_{kt}")
                stB = ring.tile([128, TQ], F32, tag="ring", name=f"stB_{p}_{kt}")
                for nch in range(2):
                    nc.tensor.matmul(
                        stA[:, nch * 512:(nch + 1) * 512],
                        lhsT=kt_sb[0:64, p, kt * 128:(kt + 1) * 128],
                        rhs=qt_sb[0:64, p, nch * 512:(nch + 1) * 512],
                        start=True,
                        stop=True,
                        tile_position=(0, 0),
                    )
                    nc.tensor.matmul(
                        stB[:, nch * 512:(nch + 1) * 512],
                        lhsT=kt_sb[64:128, p, kt * 128:(kt + 1) * 128],
                        rhs=qt_sb[64:128, p, nch * 512:(nch + 1) * 512],
                        start=True,
                        stop=True,
                        tile_position=(64, 0),
                    )
                if pend is not None:
                    ot_mm(*pend)
                ptA = ptp.tile([128, TQ], BF16, tag="pt", name=f"ptA_{p}_{kt}")
                ptB = ptp.tile([128, TQ], BF16, tag="pt", name=f"ptB_{p}_{kt}")
                nc.scalar.activation(
                    out=ptA, in_=stA,
                    func=mybir.ActivationFunctionType.Exp, scale=SCALE,
                )
                nc.scalar.activation(
                    out=ptB, in_=stB,
                    func=mybir.ActivationFunctionType.Exp, scale=SCALE,
                )
                pend = (kt, ptA, ptB)
                for f in fill_at.get(kt, ()):
                    f()
            ot_mm(*pend)

            # --------------------------------------------- pair epilogue
            # Drain the OT banks fast: one DVE copy grabs the 64 value
            # rows AND the denominator row (bf16), freeing the 4 PSUM
            # banks ~2.5us after the last OT matmul.  The reciprocal
            # (Ln/Exp on ACT), DRAM-bounce broadcast and normalize are
            # DEFERRED into the next pair's iterations so they never
            # stall the exp stream at the pair boundary.
            attU = eps.tile([65, 2, TQ], BF16, tag="attU", bufs=1,
                            name=f"attU_{p}")
            nc.vector.tensor_copy(attU, psAB[0:65, :, :])

            # The reciprocal chain is de-prioritized so the scheduler
            # slots it into ACT/DVE slack instead of blocking the exp
            # stream at the pair boundary.
            with tc.high_priority(offset=(-2000 if p < NPAIR - 1 else 0)):
                rsl = eps.tile([65, 2, TQ], F32, tag="rsl", bufs=1,
                               name=f"rsl_{p}")
                rs = eps.tile([65, 2, TQ], BF16, tag="rs", bufs=1,
                              name=f"rs_{p}")
                rbcA = eps.tile([64, TQ], BF16, tag="rbcA", bufs=1,
                                name=f"rbcA_{p}")
                rbcB = eps.tile([64, TQ], BF16, tag="rbcB", bufs=1,
                                name=f"rbcB_{p}")
                attB = eps.tile([64, TQ], BF16, tag="attB", bufs=1,
                                name=f"attB_{p}")
                nc.scalar.activation(
                    out=rsl[64:65, :, :], in_=attU[64:65, :, :],
                    func=mybir.ActivationFunctionType.Ln,
                )
                nc.scalar.activation(
                    out=rs[64:65, :, :], in_=rsl[64:65, :, :],
                    func=mybir.ActivationFunctionType.Exp, scale=-1.0,
                )
                nc.sync.dma_start(out=rs_dram[p], in_=rs[64:65, :, :])
                nc.sync.dma_start(
                    out=rbcA, in_=rs_dram[p, 0:1, :].broadcast_to([64, TQ]))
                nc.sync.dma_start(
                    out=rbcB, in_=rs_dram[p, 1:2, :].broadcast_to([64, TQ]))
                nc.vector.tensor_mul(att_sb[0:64, p, :],
                                     attU[0:64, 0, :], rbcA)
                nc.vector.tensor_mul(attB, attU[0:64, 1, :], rbcB)
                nc.sync.dma_start(out=att_sb[64:128, p, :], in_=attB)


def _phase_proj(nc, tc, wp_sb, att_sb, out_ext):
    """out = attT^T @ WpT, per 128-token tile.  Pair 7's contribution is
    deferred by one token tile so its late-normalized att never blocks
    the in-order PE queue."""
    with tc.tile_pool(name="pph", bufs=2) as pph, \
         tc.tile_pool(name="pps", bufs=2, space="PSUM") as pps:
        NTT = TQ // 128

        def head_mm(ps, tt):
            for p in range(NPAIR - 1):
                for nch in range(2):
                    nc.tensor.matmul(
                        ps[:, nch * 512:(nch + 1) * 512],
                        lhsT=att_sb[:, p, tt * 128:(tt + 1) * 128],
                        rhs=wp_sb[:, p, nch * 512:(nch + 1) * 512],
                        start=(p == 0),
                        stop=False,
                    )

        def tail_mm(ps, tt):
            for nch in range(2):
                nc.tensor.matmul(
                    ps[:, nch * 512:(nch + 1) * 512],
                    lhsT=att_sb[:, NPAIR - 1, tt * 128:(tt + 1) * 128],
                    rhs=wp_sb[:, NPAIR - 1, nch * 512:(nch + 1) * 512],
                    start=False,
                    stop=True,
                )
            o = pph.tile([128, C], F32, tag="o", name=f"o_{tt}")
            if tt % 2 == 0:
                nc.vector.tensor_copy(o, ps)
            else:
                nc.scalar.activation(
                    out=o, in_=ps, func=mybir.ActivationFunctionType.Copy
                )
            nc.sync.dma_start(out=out_ext[tt * 128:(tt + 1) * 128, :], in_=o)

        pend = None
        for tt in range(NTT):
            ps = pps.tile([128, C], F32, tag="ps", name=f"ps_{tt}")
            head_mm(ps, tt)
            if pend is not None:
                tail_mm(*pend)
            pend = (ps, tt)
        tail_mm(*pend)


_NC_CACHE = None


def _get_nc():
    global _NC_CACHE
    if _NC_CACHE is None:
        _NC_CACHE = _build_nc()
    return _NC_CACHE


# ---------------------------------------------------------------------------
# Host wrapper
# ---------------------------------------------------------------------------

def kernel(x, W_qkv, W_proj, cos, sin, mask):
    bf = ml_dtypes.bfloat16
    x = np.asarray(x, dtype=np.float32)
    W_qkv = np.asarray(W_qkv, dtype=np.float32)
    W_proj = np.asarray(W_proj, dtype=np.float32)
    cos = np.asarray(cos, dtype=np.float32)
    sin = np.asarray(sin, dtype=np.float32)

    # Permute q/k head dims: interleaved (x1,x2 pairs) -> halves [x1; x2].
    perm = np.concatenate([np.arange(0, HD, 2), np.arange(1, HD, 2)])
    Wq = W_qkv[0:C].reshape(H, HD, C)[:, perm, :].reshape(C, C)
    Wk = W_qkv[C:2 * C].reshape(H, HD, C)[:, perm, :].reshape(C, C)
    Wv = W_qkv[2 * C:3 * C]

    # per-pair tiled layouts: [NPAIR, 128 c-part, CC, 128 d]
    wqt = np.ascontiguousarray(
        Wq.T.astype(bf).reshape(CC, 128, NPAIR, 128).transpose(2, 1, 0, 3)
    )
    wkt = np.ascontiguousarray(
        Wk.T.astype(bf).reshape(CC, 128, NPAIR, 128).transpose(2, 1, 0, 3)
    )
    # V weights in 2-pair (256 v-dim) group slabs: [NVG, 128 c-part, CC, 256]
    wvg = np.ascontiguousarray(
        Wv.T.astype(bf).reshape(CC, 128, NVG, 256).transpose(2, 1, 0, 3)
    )
    wpt = np.ascontiguousarray(W_proj.T.astype(bf))

    # RoPE tables in transposed/replicated layout:
    #   cosr[r, t] = cos[t, r % 32]
    #   sinB[r, t] = +sin[t, r%32] for (r%64)<32 else -sin[t, r%32]
    cosT = cos.T
    sinT = sin.T
    cosr = np.ascontiguousarray(np.tile(cosT, (4, 1)).astype(bf))
    sinB = np.ascontiguousarray(
        np.tile(np.concatenate([sinT, -sinT], axis=0), (2, 1)).astype(bf)
    )

    in_maps = []
    for c in range(NCORES):
        b, hf = divmod(c, 2)
        qs = hf * TQ
        # token order per core: own q half first, partner half second
        # (attention is permutation-invariant over k tokens as long as
        # KT / V / rope tables all use the same order)
        ordr = np.concatenate(
            [np.arange(qs, qs + TQ), np.arange((TQ + qs) % T, (TQ + qs) % T + TQ)]
        )
        xtb = np.ascontiguousarray(x[b].T.astype(bf)[:, ordr])
        in_maps.append(
            {
                "xt": xtb,
                "wqt": wqt,
                "wkt": wkt,
                "wvg": wvg,
                "wpt": wpt,
                "cosk": np.ascontiguousarray(cosr[:, ordr]),
                "sink": np.ascontiguousarray(sinB[:, ordr]),
            }
        )

    nc = _get_nc()
    trace = bool(int(os.environ.get("BASSK_TRACE", "0")))
    res = bass_utils.run_bass_kernel_spmd(
        nc, in_maps, core_ids=list(range(NCORES)), trace=trace
    )
    if trace:
        kernel.last_exec_time_ns = res.exec_time_ns
        kernel.last_profile = res

    out = np.empty((B, T, C), dtype=np.float32)
    for c in range(NCORES):
        b, hf = divmod(c, 2)
        qs = hf * TQ
        out[b, qs:qs + TQ, :] = res.results[c]["out"]
    return out



# revision 10
# speedup vs baseline: 1.0197x; 1.0197x over previous
"""Distributed Trainium2 Bass kernel for nn_Attention_62766652063769 (v3).

Reference computation (B=4, T=2048, C=1024, H=16, HD=64):
    qkv = x @ W_qkv^T ; split into q, k, v heads
    q, k <- RoPE(q), RoPE(k)   (interleaved-pair rotation)
    attn = softmax(q k^T / sqrt(HD))   (mask is all-ones -> no masking)
    out  = (attn @ v) @ W_proj^T

Sharding: 8 cores; core c owns batch b = c//2 and query-token half c%2
(1024 q tokens).  K/V for the full 2048-token batch are computed
redundantly by both cores of a pair - zero inter-core communication.

v3 design (vs v2 at ~570us):
  - Merged score tiles: one PSUM tile [128, 1024] per (kt, q-half)
    holds BOTH heads side by side (head A cols 0:512 via
    tile_position (0,0), head B cols 512:1024 via (64,0)).  The two
    64-contraction matmuls target different PSUM banks and different
    row groups -> hardware row-tiling concurrency; and the exp for a
    tile can start after only 2 matmuls.  Ring recycle waits also
    clear a full exp earlier than the old stA/stB split.
  - Softmax denominator reciprocal moved off ACT: DMA the fp32 den
    row to DRAM, broadcast-read it to 64 partitions, DVE
    reciprocal_approx_fast + muls.  ACT now runs ONLY Exp; the
    pair-7 epilogue chain that gated the proj tail shrinks ~7us.
  - Fillers reordered: Q(p+1)/K(p+1,q0) grants are emitted FIRST in
    each pair (the old order put them last, stalling the next pair's
    first S matmul ~3.2us on the late rope add).
  - Weights prefetch two pairs ahead so iteration-0 fillers never
    wait on their weight DMA.
  - V projections merged into 512-wide matmuls (8 heads per grant).
  - OT matmuls ordered A,A,B,B for stationary reuse.
  - Prologue: xt loaded in 512-column pieces so the first Q grant
    starts ~3us earlier; V grants interleaved between startup QK
    grants to cover the rope-chain latency.
  - Proj tail: 3 PSUM bufs + depth-2 tail deferral.
"""

import os
import re
import sys
import types

if "/opt/trn_rl_repo" not in sys.path:
    sys.path.insert(0, "/opt/trn_rl_repo")

import ml_dtypes
import numpy as np

import bass_rust
import concourse.bass as bass
import concourse.mybir as mybir
from concourse import bass_utils
from concourse.tile import TileContext, ScopedClock

# ---------------------------------------------------------------------------
# Environment patches (same as v1/v2)
# ---------------------------------------------------------------------------

def _patched_drain_and_barrier(self, tick_clock, wait_clock):
    """The walrus build in this container encodes at most one sync-wait per
    instruction; Tile's tail drain carries one wait per live semaphore.
    Emit single-wait NOPs on SP instead, then an unguarded drain."""
    gc = tick_clock.global_clock
    ticks = [int(x) for x in re.findall(r"\d+", repr(gc))]
    for i, t in enumerate(ticks):
        if t <= 0:
            continue
        l = [0] * len(ticks)
        l[i] = t
        nop = self.nc.sync.nop(nofuse=True)
        wait_clock.add_sem_waits(nop.ins, ScopedClock({None: bass_rust.VectorClock(l)}))
    self.nc.sync.drain()
    self.nc.all_engine_barrier()
    assert self.sems is not None
    popped = self.nc._tile_sem_poison_stack.pop()
    assert popped is self._sem_poison
    self.nc.clear_and_free_semaphores(list(self.sems.allocated().values()))
    self.nc.all_engine_barrier()


TileContext._drain_and_barrier = _patched_drain_and_barrier


def _split_multi_waits(nc):
    """Move extra sync-waits onto single-wait NOPs inserted just before the
    owning instruction on the same (in-order) engine."""
    for func in nc.m.functions:
        for bb in func.blocks:
            insts = bb.instructions
            if not any(
                i.sync_info is not None
                and i.sync_info.on_wait
                and len(i.sync_info.on_wait) > 1
                for i in insts
            ):
                continue
            new = []
            for inst in insts:
                si = inst.sync_info
                if si is not None and si.on_wait and len(si.on_wait) > 1:
                    waits = list(si.on_wait)
                    for w in waits[:-1]:
                        nop = mybir.InstNoOp(
                            name=nc.get_next_instruction_name(),
                            engine=inst.engine,
                            bass_nofuse=True,
                            sync_info=mybir.SyncInfo(on_wait=[w], on_update=[]),
                        )
                        nc.register_instruction(nop)
                        new.append(nop)
                    inst.sync_info = mybir.SyncInfo(
                        on_wait=[waits[-1]], on_update=list(si.on_update)
                    )
                new.append(inst)
            bb.instructions = new


def _install_ntff_hook():
    """Recreate antenv.axon_hooks (absent in this image) so
    run_bass_kernel_spmd(trace=True) can profile through libaxon_pjrt."""
    if "antenv.axon_hooks" in sys.modules:
        return
    import contextlib
    import ctypes

    mod = types.ModuleType("antenv.axon_hooks")
    _state = {"hook": None}

    def set_axon_ntff_profile_hook(hook):
        _state["hook"] = hook

    def get_axon_ntff_profile_hook():
        return _state["hook"]

    def _ntff_profile_via_ctypes(so_path):
        lib = ctypes.CDLL(so_path)
        if not hasattr(lib, "axon_start_nrt_profile"):
            return None
        lib.axon_start_nrt_profile.argtypes = [
            ctypes.POINTER(ctypes.c_int64),
            ctypes.c_size_t,
        ]
        lib.axon_start_nrt_profile.restype = ctypes.c_int64
        lib.axon_stop_nrt_profile.argtypes = [ctypes.c_char_p]
        lib.axon_stop_nrt_profile.restype = ctypes.c_int64

        @contextlib.contextmanager
        def _hook(output_dir, device_ids):
            import jax

            jax.devices()
            if device_ids:
                ids = (ctypes.c_int64 * len(device_ids))(*device_ids)
                rc = lib.axon_start_nrt_profile(ids, len(device_ids))
            else:
                rc = lib.axon_start_nrt_profile(None, 0)
            if rc != 0:
                raise RuntimeError(f"axon_start_nrt_profile rc={rc}")
            try:
                yield
            finally:
                n = lib.axon_stop_nrt_profile(str(output_dir).encode())
                if n < 0:
                    raise RuntimeError(f"axon_stop_nrt_profile rc={n}")
                print(f"profile: {n} file(s) in {output_dir}", file=sys.stderr)

        return _hook

    mod.set_axon_ntff_profile_hook = set_axon_ntff_profile_hook
    mod.get_axon_ntff_profile_hook = get_axon_ntff_profile_hook
    try:
        set_axon_ntff_profile_hook(
            _ntff_profile_via_ctypes("/opt/axon/libaxon_pjrt.so")
        )
    except Exception:
        pass
    sys.modules["antenv.axon_hooks"] = mod
    try:
        import antenv

        antenv.axon_hooks = mod
    except ImportError:
        pass


_install_ntff_hook()

# ---------------------------------------------------------------------------
# Problem constants
# ---------------------------------------------------------------------------

B, T, C = 4, 2048, 1024
H, HD = 16, 64
NCORES = 8
TQ = T // 2          # q tokens per core
NPAIR = H // 2       # head pairs (=8); pair p holds heads 2p, 2p+1
NVG = 2              # V groups of 4 pairs (8 heads, 512 v dims)
KT_TILES = T // 128  # 16
SCALE = 1.0 / np.sqrt(HD)

F32 = mybir.dt.float32
BF16 = mybir.dt.bfloat16

CC = C // 128  # 8 contraction chunks


# ---------------------------------------------------------------------------
# Device program
# ---------------------------------------------------------------------------

def _build_nc():
    nc = bass.Bass(trn_type="TRN2", target_bir_lowering=False, debug=False)

    xt = nc.declare_dram_parameter("xt", [C, T], BF16, isOutput=False)
    wqt = nc.declare_dram_parameter("wqt", [NPAIR, 128, CC, 128], BF16,
                                    isOutput=False)
    wkt = nc.declare_dram_parameter("wkt", [NPAIR, 128, CC, 128], BF16,
                                    isOutput=False)
    wvg = nc.declare_dram_parameter("wvg", [NVG, 128, CC, 512], BF16,
                                    isOutput=False)
    wpt = nc.declare_dram_parameter("wpt", [C, C], BF16, isOutput=False)
    cosk = nc.declare_dram_parameter("cosk", [128, T], BF16, isOutput=False)
    sink = nc.declare_dram_parameter("sink", [128, T], BF16, isOutput=False)
    out_ext = nc.declare_dram_parameter("out", [TQ, C], F32, isOutput=True)

    rs_dram = nc.dram_tensor("rs_scratch", [NPAIR, 2, TQ], BF16)

    with TileContext(nc) as tc:
        with tc.tile_pool(name="persist", bufs=1) as persist:
            qt_sb = persist.tile([128, NPAIR, TQ], BF16, tag="qt")
            att_sb = persist.tile([128, NPAIR, TQ], BF16, tag="att")
            kt_sb = persist.tile([128, NPAIR, T], BF16, tag="kt")
            v_sb = persist.tile([128, KT_TILES, H, 65], BF16, tag="v")
            ck = persist.tile([128, T], BF16, tag="ck")
            sk = persist.tile([128, T], BF16, tag="sk")

            with tc.tile_pool(name="xtpool", bufs=1) as xtpool, \
                 tc.tile_pool(name="pw", bufs=1) as pw:
                xt_sb = xtpool.tile([128, CC, T], BF16, tag="xt")
                xt_r = xt.rearrange("(cc p) t -> p cc t", p=128)
                nc.vector.memset(v_sb[:, :, :, 64:65], 1.0)

                wp_sb = pw.tile([128, CC, C], BF16)

                _attention(nc, tc, xt_sb, qt_sb, kt_sb, v_sb, att_sb,
                           ck, sk, wqt, wkt, wvg, rs_dram, wp_sb, wpt,
                           xt_r, cosk, sink)

                _phase_proj(nc, tc, wp_sb, att_sb, out_ext)

    _split_multi_waits(nc)
    return nc


def _attention(nc, tc, xt_sb, qt_sb, kt_sb, v_sb, att_sb, ck, sk,
               wqt, wkt, wvg, rs_dram, wp_sb, wpt, xt_r, cosk, sink):
    with tc.tile_pool(name="ring", bufs=2, space="PSUM") as ring, \
         tc.tile_pool(name="otps", bufs=1, space="PSUM") as otps, \
         tc.tile_pool(name="wts", bufs=2) as wts, \
         tc.tile_pool(name="qkx", bufs=2) as qkx, \
         tc.tile_pool(name="ptp", bufs=4) as ptp, \
         tc.tile_pool(name="eps", bufs=1) as eps:

        wq_tiles = {}
        wk_tiles = {}
        wv_tiles = {}

        def load_wq(p):
            t = wts.tile([128, CC, 128], BF16, tag="wq")
            nc.gpsimd.dma_start(out=t, in_=wqt[p])
            wq_tiles[p] = t

        def load_wk(p):
            t = wts.tile([128, CC, 128], BF16, tag="wk")
            nc.gpsimd.dma_start(out=t, in_=wkt[p])
            wk_tiles[p] = t

        def load_wv(g):
            t = wts.tile([128, CC, 512], BF16, tag="wv", bufs=1)
            nc.gpsimd.dma_start(out=t, in_=wvg[g])
            wv_tiles[g] = t

        def qk_grant(dst_sb, pair, wtile, col0, swap_eng=None):
            """One 512-token-column projection + rope grant -> dst_sb."""
            if swap_eng is None:
                swap_eng = nc.sync
            ps = ring.tile([128, TQ], F32, tag="ring", name=f"ps_{pair}_{col0}")
            for cc in range(CC):
                nc.tensor.matmul(
                    ps[:, 0:512],
                    lhsT=wtile[:, cc, :],
                    rhs=xt_sb[:, cc, col0:col0 + 512],
                    start=(cc == 0),
                    stop=(cc == CC - 1),
                )
            xb = qkx.tile([128, 512], BF16, tag="xb", name=f"xb_{pair}_{col0}")
            nc.vector.tensor_copy(xb, ps[:, 0:512])
            u = qkx.tile([128, 512], BF16, tag="u", name=f"u_{pair}_{col0}")
            v = qkx.tile([128, 512], BF16, tag="v", name=f"v_{pair}_{col0}")
            vs = qkx.tile([128, 512], BF16, tag="vs", name=f"vs_{pair}_{col0}")
            nc.vector.tensor_mul(u, xb, ck[:, col0:col0 + 512])
            nc.vector.tensor_mul(v, xb, sk[:, col0:col0 + 512])
            for blk in range(4):
                r = blk * 32
                s = (blk ^ 1) * 32
                swap_eng.dma_start(out=vs[r:r + 32, :], in_=v[s:s + 32, :])
            nc.gpsimd.tensor_add(dst_sb[:, pair, col0:col0 + 512], u, vs)

        def v_grant(g, tt):
            """V columns for group g (8 heads), token tile tt."""
            psv = ring.tile([128, 512], F32, tag="ring", name=f"psv_{g}_{tt}")
            for cc in range(CC):
                nc.tensor.matmul(
                    psv,
                    lhsT=xt_sb[:, cc, tt * 128:(tt + 1) * 128],
                    rhs=wv_tiles[g][:, cc, :],
                    start=(cc == 0),
                    stop=(cc == CC - 1),
                )
            nc.vector.tensor_copy(
                v_sb[:, tt, 8 * g:8 * g + 8, 0:64],
                psv.rearrange("p (h d) -> p h d", h=8),
            )

        # ------------------------------------------------------- startup
        # xt arrives in 512-column pieces so the first Q grant starts as
        # soon as ~1.5MB (wq + one piece) has landed.  V grants are
        # interleaved between the QK grants to keep the PE busy during
        # the rope chains.
        load_wq(0)
        nc.sync.dma_start(out=xt_sb[:, :, 0:512], in_=xt_r[:, :, 0:512])
        load_wk(0)
        load_wv(0)
        nc.scalar.dma_start(out=ck[:, 0:512], in_=cosk[:, 0:512])
        nc.scalar.dma_start(out=sk[:, 0:512], in_=sink[:, 0:512])
        load_wq(1)
        load_wk(1)
        nc.sync.dma_start(out=xt_sb[:, :, 512:1024], in_=xt_r[:, :, 512:1024])
        nc.scalar.dma_start(out=ck[:, 512:1024], in_=cosk[:, 512:1024])
        nc.scalar.dma_start(out=sk[:, 512:1024], in_=sink[:, 512:1024])
        qk_grant(qt_sb, 0, wq_tiles[0], 0, swap_eng=nc.scalar)
        v_grant(0, 0)
        v_grant(0, 1)
        qk_grant(qt_sb, 0, wq_tiles[0], 512, swap_eng=nc.scalar)
        v_grant(0, 2)
        v_grant(0, 3)
        qk_grant(kt_sb, 0, wk_tiles[0], 0, swap_eng=nc.scalar)
        nc.sync.dma_start(out=xt_sb[:, :, 1024:1536], in_=xt_r[:, :, 1024:1536])
        nc.sync.dma_start(out=xt_sb[:, :, 1536:2048], in_=xt_r[:, :, 1536:2048])
        nc.scalar.dma_start(out=ck[:, TQ:T], in_=cosk[:, TQ:T])
        nc.scalar.dma_start(out=sk[:, TQ:T], in_=sink[:, TQ:T])

        # ------------------------------------------------- attention loop
        for p in range(NPAIR):
            if p < 6:
                load_wq(p + 2)
                load_wk(p + 2)
            if p == 1:
                # wv is single-buffered (SBUF pressure): group 1 loads
                # into group 0's slot once pair 0's V grants are done.
                load_wv(1)
            if p == 6:
                # preload W_proj so the proj phase starts immediately
                wp_r = wpt.rearrange("(cc p) e -> p cc e", p=128)
                nc.sync.dma_start(out=wp_sb[:, 0:4, :], in_=wp_r[:, 0:4, :])
                nc.sync.dma_start(out=wp_sb[:, 4:8, :], in_=wp_r[:, 4:8, :])

            def mk_v(g, tt):
                return lambda: v_grant(g, tt)

            def mk_k(q, n):
                return lambda: qk_grant(kt_sb, q, wk_tiles[q], n * 512)

            def mk_q(q, n):
                return lambda: qk_grant(qt_sb, q, wq_tiles[q], n * 512)

            if p == 0:
                # Prefill.  V(0, tt) must be emitted at iteration <= tt
                # (OT(kt=tt) runs at iteration tt+1); K0 quarter n before
                # iteration 4n.  Next-pair Q/K grants ride the back half.
                v_grant(0, 4)
                fill_at = {
                    0: [mk_v(0, 5)],
                    1: [mk_v(0, 6), mk_k(0, 1)],
                    2: [mk_v(0, 7)],
                    3: [mk_v(0, 8)],
                    4: [mk_v(0, 9)],
                    5: [mk_v(0, 10), mk_k(0, 2)],
                    6: [mk_v(0, 11)],
                    7: [mk_v(0, 12)],
                    8: [mk_v(0, 13), mk_q(1, 0)],
                    9: [mk_v(0, 14), mk_k(0, 3)],
                    10: [mk_v(0, 15), mk_q(1, 1)],
                    11: [mk_k(1, 0)],
                    12: [mk_k(1, 1)],
                    14: [mk_k(1, 2)],
                    15: [mk_k(1, 3)],
                }
            else:
                # Q(p+1) first: the next pair's first S matmul waits on
                # Q's rope add, so it must complete well before the
                # boundary.  K quarters n are needed by iteration 4n of
                # pair p+1.  V group 1 grants (needed from pair 4,
                # token-tile tt by its iteration tt) spread over pairs
                # 1-4.
                fill_at = {}
                if p < 7:
                    fill_at[0] = [mk_q(p + 1, 0)]
                    fill_at[1] = [mk_q(p + 1, 1)]
                    fill_at[2] = [mk_k(p + 1, 0)]
                    fill_at[4] = [mk_k(p + 1, 1)]
                    fill_at[10] = [mk_k(p + 1, 2)]
                    fill_at[12] = [mk_k(p + 1, 3)]
                if 1 <= p <= 4:
                    vtt = range((p - 1) * 4, (p - 1) * 4 + 4)
                    for slot, tt in zip((3, 5, 7, 9), vtt):
                        fill_at.setdefault(slot, []).append(mk_v(1, tt))

            psAB = otps.tile([128, 2, TQ], F32, tag="ot", name=f"psAB_{p}")

            def ot_mm(kt, pt0, pt1):
                # pt0/pt1 hold [head A q-chunk | head B q-chunk] for
                # q cols 0:512 / 512:1024.  A,A,B,B order reuses the
                # stationary V tile across the two q chunks.
                nc.tensor.matmul(
                    psAB[0:65, 0, 0:512],
                    lhsT=v_sb[:, kt, 2 * p, :],
                    rhs=pt0[:, 0:512],
                    start=(kt == 0),
                    stop=(kt == KT_TILES - 1),
                )
                nc.tensor.matmul(
                    psAB[0:65, 0, 512:1024],
                    lhsT=v_sb[:, kt, 2 * p, :],
                    rhs=pt1[:, 0:512],
                    start=(kt == 0),
                    stop=(kt == KT_TILES - 1),
                )
                nc.tensor.matmul(
                    psAB[0:65, 1, 0:512],
                    lhsT=v_sb[:, kt, 2 * p + 1, :],
                    rhs=pt0[:, 512:1024],
                    start=(kt == 0),
                    stop=(kt == KT_TILES - 1),
                )
                nc.tensor.matmul(
                    psAB[0:65, 1, 512:1024],
                    lhsT=v_sb[:, kt, 2 * p + 1, :],
                    rhs=pt1[:, 512:1024],
                    start=(kt == 0),
                    stop=(kt == KT_TILES - 1),
                )

            # Software-pipelined: the PE is in-order, so OT(kt) must not
            # be enqueued before S(kt+1) — it would stall the whole PE
            # queue on exp(kt).  Defer OT by one kt.
            pend = None
            for kt in range(KT_TILES):
                # Merged score tile: head A -> cols 0:512 (rows 0:64 of
                # the array), head B -> cols 512:1024 (rows 64:128).
                # The two matmuls hit different PSUM banks and row
                # groups -> they can run concurrently.
                st0 = ring.tile([128, TQ], F32, tag="ring",
                                name=f"st0_{p}_{kt}")
                st1 = ring.tile([128, TQ], F32, tag="ring",
                                name=f"st1_{p}_{kt}")
                nc.tensor.matmul(
                    st0[:, 0:512],
                    lhsT=kt_sb[0:64, p, kt * 128:(kt + 1) * 128],
                    rhs=qt_sb[0:64, p, 0:512],
                    start=True, stop=True, tile_position=(0, 0),
                )
                nc.tensor.matmul(
                    st0[:, 512:1024],
                    lhsT=kt_sb[64:128, p, kt * 128:(kt + 1) * 128],
                    rhs=qt_sb[64:128, p, 0:512],
                    start=True, stop=True, tile_position=(64, 0),
                )
                pt0 = ptp.tile([128, TQ], BF16, tag="pt",
                               name=f"pt0_{p}_{kt}")
                nc.scalar.activation(
                    out=pt0, in_=st0,
                    func=mybir.ActivationFunctionType.Exp, scale=SCALE,
                )
                nc.tensor.matmul(
                    st1[:, 0:512],
                    lhsT=kt_sb[0:64, p, kt * 128:(kt + 1) * 128],
                    rhs=qt_sb[0:64, p, 512:1024],
                    start=True, stop=True, tile_position=(0, 0),
                )
                nc.tensor.matmul(
                    st1[:, 512:1024],
                    lhsT=kt_sb[64:128, p, kt * 128:(kt + 1) * 128],
                    rhs=qt_sb[64:128, p, 512:1024],
                    start=True, stop=True, tile_position=(64, 0),
                )
                pt1 = ptp.tile([128, TQ], BF16, tag="pt",
                               name=f"pt1_{p}_{kt}")
                nc.scalar.activation(
                    out=pt1, in_=st1,
                    func=mybir.ActivationFunctionType.Exp, scale=SCALE,
                )
                if pend is not None:
                    ot_mm(*pend)
                pend = (kt, pt0, pt1)
                for f in fill_at.get(kt, ()):
                    f()
            ot_mm(*pend)

            # --------------------------------------------- pair epilogue
            # Value rows -> bf16 SBUF (frees most of psAB); the fp32
            # denominator row DMAs to DRAM, broadcast-reads back onto 64
            # partitions, and the reciprocal+normalize run on the DVE.
            # ACT stays exp-only all kernel.
            attU = eps.tile([65, 2, TQ], BF16, tag="attU", bufs=1,
                            name=f"attU_{p}")
            nc.vector.tensor_copy(attU, psAB[0:65, :, :])
            nc.sync.dma_start(out=rs_dram[p], in_=attU[64:65, :, :])
            rbc = eps.tile([64, 2, TQ], BF16, tag="rbc", bufs=1,
                           name=f"rbc_{p}")
            nc.sync.dma_start(
                out=rbc[:, 0, :],
                in_=rs_dram[p, 0:1, :].broadcast_to([64, TQ]))
            nc.sync.dma_start(
                out=rbc[:, 1, :],
                in_=rs_dram[p, 1:2, :].broadcast_to([64, TQ]))
            attB = eps.tile([64, TQ], BF16, tag="attB", bufs=1,
                            name=f"attB_{p}")
            for h in range(2):
                rf = eps.tile([64, TQ], F32, tag="rf", bufs=1,
                              name=f"rf_{p}_{h}")
                rg = eps.tile([64, TQ], F32, tag="rg", bufs=1,
                              name=f"rg_{p}_{h}")
                nc.vector.tensor_copy(rf, rbc[:, h, :])
                nc.vector.reciprocal(rg, rf)
                if h == 0:
                    nc.vector.tensor_mul(att_sb[0:64, p, :],
                                         attU[0:64, 0, :], rg)
                else:
                    nc.vector.tensor_mul(attB, attU[0:64, 1, :], rg)
            nc.sync.dma_start(out=att_sb[64:128, p, :], in_=attB)


def _phase_proj(nc, tc, wp_sb, att_sb, out_ext):
    """out = attT^T @ WpT, per 128-token tile.  Pair 7's contribution is
    deferred by two token tiles so its late-normalized att never blocks
    the in-order PE queue."""
    with tc.tile_pool(name="pph", bufs=2) as pph, \
         tc.tile_pool(name="pps", bufs=3, space="PSUM") as pps:
        NTT = TQ // 128

        def head_mm(ps, tt):
            for p in range(NPAIR - 1):
                for nch in range(2):
                    nc.tensor.matmul(
                        ps[:, nch * 512:(nch + 1) * 512],
                        lhsT=att_sb[:, p, tt * 128:(tt + 1) * 128],
                        rhs=wp_sb[:, p, nch * 512:(nch + 1) * 512],
                        start=(p == 0),
                        stop=False,
                    )

        def tail_mm(ps, tt):
            for nch in range(2):
                nc.tensor.matmul(
                    ps[:, nch * 512:(nch + 1) * 512],
                    lhsT=att_sb[:, NPAIR - 1, tt * 128:(tt + 1) * 128],
                    rhs=wp_sb[:, NPAIR - 1, nch * 512:(nch + 1) * 512],
                    start=False,
                    stop=True,
                )
            o = pph.tile([128, C], F32, tag="o", name=f"o_{tt}")
            if tt % 2 == 0:
                nc.vector.tensor_copy(o, ps)
            else:
                nc.scalar.activation(
                    out=o, in_=ps, func=mybir.ActivationFunctionType.Copy
                )
            nc.sync.dma_start(out=out_ext[tt * 128:(tt + 1) * 128, :], in_=o)

        pend = []
        for tt in range(NTT):
            ps = pps.tile([128, C], F32, tag="ps", name=f"ps_{tt}")
            head_mm(ps, tt)
            pend.append((ps, tt))
            if len(pend) > 2:
                tail_mm(*pend.pop(0))
        for e in pend:
            tail_mm(*e)


_NC_CACHE = None


def _get_nc():
    global _NC_CACHE
    if _NC_CACHE is None:
        _NC_CACHE = _build_nc()
    return _NC_CACHE


# ---------------------------------------------------------------------------
# Host wrapper
# ---------------------------------------------------------------------------

def kernel(x, W_qkv, W_proj, cos, sin, mask):
    bf = ml_dtypes.bfloat16
    x = np.asarray(x, dtype=np.float32)
    W_qkv = np.asarray(W_qkv, dtype=np.float32)
    W_proj = np.asarray(W_proj, dtype=np.float32)
    cos = np.asarray(cos, dtype=np.float32)
    sin = np.asarray(sin, dtype=np.float32)

    # Permute q/k head dims: interleaved (x1,x2 pairs) -> halves [x1; x2].
    perm = np.concatenate([np.arange(0, HD, 2), np.arange(1, HD, 2)])
    Wq = W_qkv[0:C].reshape(H, HD, C)[:, perm, :].reshape(C, C)
    Wk = W_qkv[C:2 * C].reshape(H, HD, C)[:, perm, :].reshape(C, C)
    Wv = W_qkv[2 * C:3 * C]

    # per-pair tiled layouts: [NPAIR, 128 c-part, CC, 128 d]
    wqt = np.ascontiguousarray(
        Wq.T.astype(bf).reshape(CC, 128, NPAIR, 128).transpose(2, 1, 0, 3)
    )
    wkt = np.ascontiguousarray(
        Wk.T.astype(bf).reshape(CC, 128, NPAIR, 128).transpose(2, 1, 0, 3)
    )
    # V weights in 4-pair (512 v-dim) group slabs: [NVG, 128 c-part, CC, 512]
    wvg = np.ascontiguousarray(
        Wv.T.astype(bf).reshape(CC, 128, NVG, 512).transpose(2, 1, 0, 3)
    )
    wpt = np.ascontiguousarray(W_proj.T.astype(bf))

    # RoPE tables in transposed/replicated layout:
    #   cosr[r, t] = cos[t, r % 32]
    #   sinB[r, t] = +sin[t, r%32] for (r%64)<32 else -sin[t, r%32]
    cosT = cos.T
    sinT = sin.T
    cosr = np.ascontiguousarray(np.tile(cosT, (4, 1)).astype(bf))
    sinB = np.ascontiguousarray(
        np.tile(np.concatenate([sinT, -sinT], axis=0), (2, 1)).astype(bf)
    )

    in_maps = []
    for c in range(NCORES):
        b, hf = divmod(c, 2)
        qs = hf * TQ
        # token order per core: own q half first, partner half second
        # (attention is permutation-invariant over k tokens as long as
        # KT / V / rope tables all use the same order)
        ordr = np.concatenate(
            [np.arange(qs, qs + TQ), np.arange((TQ + qs) % T, (TQ + qs) % T + TQ)]
        )
        xtb = np.ascontiguousarray(x[b].T.astype(bf)[:, ordr])
        in_maps.append(
            {
                "xt": xtb,
                "wqt": wqt,
                "wkt": wkt,
                "wvg": wvg,
                "wpt": wpt,
                "cosk": np.ascontiguousarray(cosr[:, ordr]),
                "sink": np.ascontiguousarray(sinB[:, ordr]),
            }
        )

    nc = _get_nc()
    trace = bool(int(os.environ.get("BASSK_TRACE", "0")))
    res = bass_utils.run_bass_kernel_spmd(
        nc, in_maps, core_ids=list(range(NCORES)), trace=trace
    )
    if trace:
        kernel.last_exec_time_ns = res.exec_time_ns
        kernel.last_profile = res

    out = np.empty((B, T, C), dtype=np.float32)
    for c in range(NCORES):
        b, hf = divmod(c, 2)
        qs = hf * TQ
        out[b, qs:qs + TQ, :] = res.results[c]["out"]
    return out


# revision 12
# speedup vs baseline: 1.2090x; 1.1856x over previous
"""Distributed Trainium2 Bass kernel for nn_Attention_62766652063769 (v3).

Reference computation (B=4, T=2048, C=1024, H=16, HD=64):
    qkv = x @ W_qkv^T ; split into q, k, v heads
    q, k <- RoPE(q), RoPE(k)   (interleaved-pair rotation)
    attn = softmax(q k^T / sqrt(HD))   (mask is all-ones -> no masking)
    out  = (attn @ v) @ W_proj^T

Sharding: 8 cores; core c owns batch b = c//2 and query-token half c%2
(1024 q tokens).  K/V for the full 2048-token batch are computed
redundantly by both cores of a pair - zero inter-core communication.

v3 design (vs v2 at ~570us):
  - Merged score tiles: one PSUM tile [128, 1024] per (kt, q-half)
    holds BOTH heads side by side (head A cols 0:512 via
    tile_position (0,0), head B cols 512:1024 via (64,0)).  The two
    64-contraction matmuls target different PSUM banks and different
    row groups -> hardware row-tiling concurrency; and the exp for a
    tile can start after only 2 matmuls.  Ring recycle waits also
    clear a full exp earlier than the old stA/stB split.
  - Softmax denominator reciprocal moved off ACT: DMA the fp32 den
    row to DRAM, broadcast-read it to 64 partitions, DVE
    reciprocal_approx_fast + muls.  ACT now runs ONLY Exp; the
    pair-7 epilogue chain that gated the proj tail shrinks ~7us.
  - Fillers reordered: Q(p+1)/K(p+1,q0) grants are emitted FIRST in
    each pair (the old order put them last, stalling the next pair's
    first S matmul ~3.2us on the late rope add).
  - Weights prefetch two pairs ahead so iteration-0 fillers never
    wait on their weight DMA.
  - V projections merged into 512-wide matmuls (8 heads per grant).
  - OT matmuls ordered A,A,B,B for stationary reuse.
  - Prologue: xt loaded in 512-column pieces so the first Q grant
    starts ~3us earlier; V grants interleaved between startup QK
    grants to cover the rope-chain latency.
  - Proj tail: 3 PSUM bufs + depth-2 tail deferral.
"""

import os
import re
import sys
import types

if "/opt/trn_rl_repo" not in sys.path:
    sys.path.insert(0, "/opt/trn_rl_repo")

import ml_dtypes
import numpy as np

import bass_rust
import concourse.bass as bass
import concourse.mybir as mybir
from concourse import bass_utils
from concourse.tile import TileContext, ScopedClock

# ---------------------------------------------------------------------------
# Environment patches (same as v1/v2)
# ---------------------------------------------------------------------------

def _patched_drain_and_barrier(self, tick_clock, wait_clock):
    """The walrus build in this container encodes at most one sync-wait per
    instruction; Tile's tail drain carries one wait per live semaphore.
    Emit single-wait NOPs on SP instead, then an unguarded drain."""
    gc = tick_clock.global_clock
    ticks = [int(x) for x in re.findall(r"\d+", repr(gc))]
    for i, t in enumerate(ticks):
        if t <= 0:
            continue
        l = [0] * len(ticks)
        l[i] = t
        nop = self.nc.sync.nop(nofuse=True)
        wait_clock.add_sem_waits(nop.ins, ScopedClock({None: bass_rust.VectorClock(l)}))
    self.nc.sync.drain()
    self.nc.all_engine_barrier()
    assert self.sems is not None
    popped = self.nc._tile_sem_poison_stack.pop()
    assert popped is self._sem_poison
    self.nc.clear_and_free_semaphores(list(self.sems.allocated().values()))
    self.nc.all_engine_barrier()


TileContext._drain_and_barrier = _patched_drain_and_barrier


def _split_multi_waits(nc):
    """Move extra sync-waits onto single-wait NOPs inserted just before the
    owning instruction on the same (in-order) engine."""
    for func in nc.m.functions:
        for bb in func.blocks:
            insts = bb.instructions
            if not any(
                i.sync_info is not None
                and i.sync_info.on_wait
                and len(i.sync_info.on_wait) > 1
                for i in insts
            ):
                continue
            new = []
            for inst in insts:
                si = inst.sync_info
                if si is not None and si.on_wait and len(si.on_wait) > 1:
                    waits = list(si.on_wait)
                    for w in waits[:-1]:
                        nop = mybir.InstNoOp(
                            name=nc.get_next_instruction_name(),
                            engine=inst.engine,
                            bass_nofuse=True,
                            sync_info=mybir.SyncInfo(on_wait=[w], on_update=[]),
                        )
                        nc.register_instruction(nop)
                        new.append(nop)
                    inst.sync_info = mybir.SyncInfo(
                        on_wait=[waits[-1]], on_update=list(si.on_update)
                    )
                new.append(inst)
            bb.instructions = new


def _install_ntff_hook():
    """Recreate antenv.axon_hooks (absent in this image) so
    run_bass_kernel_spmd(trace=True) can profile through libaxon_pjrt."""
    if "antenv.axon_hooks" in sys.modules:
        return
    import contextlib
    import ctypes

    mod = types.ModuleType("antenv.axon_hooks")
    _state = {"hook": None}

    def set_axon_ntff_profile_hook(hook):
        _state["hook"] = hook

    def get_axon_ntff_profile_hook():
        return _state["hook"]

    def _ntff_profile_via_ctypes(so_path):
        lib = ctypes.CDLL(so_path)
        if not hasattr(lib, "axon_start_nrt_profile"):
            return None
        lib.axon_start_nrt_profile.argtypes = [
            ctypes.POINTER(ctypes.c_int64),
            ctypes.c_size_t,
        ]
        lib.axon_start_nrt_profile.restype = ctypes.c_int64
        lib.axon_stop_nrt_profile.argtypes = [ctypes.c_char_p]
        lib.axon_stop_nrt_profile.restype = ctypes.c_int64

        @contextlib.contextmanager
        def _hook(output_dir, device_ids):
            import jax

            jax.devices()
            if device_ids:
                ids = (ctypes.c_int64 * len(device_ids))(*device_ids)
                rc = lib.axon_start_nrt_profile(ids, len(device_ids))
            else:
                rc = lib.axon_start_nrt_profile(None, 0)
            if rc != 0:
                raise RuntimeError(f"axon_start_nrt_profile rc={rc}")
            try:
                yield
            finally:
                n = lib.axon_stop_nrt_profile(str(output_dir).encode())
                if n < 0:
                    raise RuntimeError(f"axon_stop_nrt_profile rc={n}")
                print(f"profile: {n} file(s) in {output_dir}", file=sys.stderr)

        return _hook

    mod.set_axon_ntff_profile_hook = set_axon_ntff_profile_hook
    mod.get_axon_ntff_profile_hook = get_axon_ntff_profile_hook
    try:
        set_axon_ntff_profile_hook(
            _ntff_profile_via_ctypes("/opt/axon/libaxon_pjrt.so")
        )
    except Exception:
        pass
    sys.modules["antenv.axon_hooks"] = mod
    try:
        import antenv

        antenv.axon_hooks = mod
    except ImportError:
        pass


_install_ntff_hook()

# ---------------------------------------------------------------------------
# Problem constants
# ---------------------------------------------------------------------------

B, T, C = 4, 2048, 1024
H, HD = 16, 64
NCORES = 8
TQ = T // 2          # q tokens per core
NPAIR = H // 2       # head pairs (=8); pair p holds heads 2p, 2p+1
NVG = 2              # V groups of 4 pairs (8 heads, 512 v dims)
KT_TILES = T // 128  # 16
SCALE = 1.0 / np.sqrt(HD)

F32 = mybir.dt.float32
BF16 = mybir.dt.bfloat16

CC = C // 128  # 8 contraction chunks


# ---------------------------------------------------------------------------
# Device program
# ---------------------------------------------------------------------------

def _build_nc():
    nc = bass.Bass(trn_type="TRN2", target_bir_lowering=False, debug=False)

    xt = nc.declare_dram_parameter("xt", [C, T], BF16, isOutput=False)
    wqt = nc.declare_dram_parameter("wqt", [NPAIR, 128, CC, 128], BF16,
                                    isOutput=False)
    wkt = nc.declare_dram_parameter("wkt", [NPAIR, 128, CC, 128], BF16,
                                    isOutput=False)
    wvg = nc.declare_dram_parameter("wvg", [NVG, 128, CC, 512], BF16,
                                    isOutput=False)
    wpt = nc.declare_dram_parameter("wpt", [C, C], BF16, isOutput=False)
    cosk = nc.declare_dram_parameter("cosk", [128, T], BF16, isOutput=False)
    sink = nc.declare_dram_parameter("sink", [128, T], BF16, isOutput=False)
    out_ext = nc.declare_dram_parameter("out", [TQ, C], F32, isOutput=True)

    rs_dram = nc.dram_tensor("rs_scratch", [NPAIR, 2, TQ], BF16)
    rc_dram = nc.dram_tensor("rc_scratch", [NPAIR, 2, TQ], BF16)

    with TileContext(nc) as tc:
        with tc.tile_pool(name="persist", bufs=1) as persist:
            qt_sb = persist.tile([128, NPAIR, TQ], BF16, tag="qt")
            att_sb = persist.tile([128, NPAIR, TQ], BF16, tag="att")
            kt_sb = persist.tile([128, NPAIR, T], BF16, tag="kt")
            v_sb = persist.tile([128, KT_TILES, H, 65], BF16, tag="v")
            ck = persist.tile([128, T], BF16, tag="ck")
            sk = persist.tile([128, T], BF16, tag="sk")

            with tc.tile_pool(name="xtpool", bufs=1) as xtpool, \
                 tc.tile_pool(name="pw", bufs=1) as pw:
                xt_sb = xtpool.tile([128, CC, T], BF16, tag="xt")
                xt_r = xt.rearrange("(cc p) t -> p cc t", p=128)
                nc.vector.memset(v_sb[:, :, :, 64:65], 1.0)

                wp_sb = pw.tile([128, CC, C], BF16)

                _attention(nc, tc, xt_sb, qt_sb, kt_sb, v_sb, att_sb,
                           ck, sk, wqt, wkt, wvg, rs_dram, rc_dram,
                           wp_sb, wpt, xt_r, cosk, sink)

                _phase_proj(nc, tc, wp_sb, att_sb, out_ext)

    _split_multi_waits(nc)
    return nc


def _attention(nc, tc, xt_sb, qt_sb, kt_sb, v_sb, att_sb, ck, sk,
               wqt, wkt, wvg, rs_dram, rc_dram, wp_sb, wpt, xt_r, cosk,
               sink):
    with tc.tile_pool(name="ring", bufs=2, space="PSUM") as ring, \
         tc.tile_pool(name="otps", bufs=1, space="PSUM") as otps, \
         tc.tile_pool(name="wts", bufs=2) as wts, \
         tc.tile_pool(name="qkx", bufs=2) as qkx, \
         tc.tile_pool(name="ptp", bufs=6) as ptp, \
         tc.tile_pool(name="eps", bufs=1) as eps:

        wq_tiles = {}
        wk_tiles = {}
        wv_tiles = {}

        def load_wq(p):
            t = wts.tile([128, CC, 128], BF16, tag="wq")
            nc.gpsimd.dma_start(out=t, in_=wqt[p])
            wq_tiles[p] = t

        def load_wk(p):
            t = wts.tile([128, CC, 128], BF16, tag="wk")
            nc.gpsimd.dma_start(out=t, in_=wkt[p])
            wk_tiles[p] = t

        def load_wv(g):
            t = wts.tile([128, CC, 512], BF16, tag="wv", bufs=1)
            nc.gpsimd.dma_start(out=t, in_=wvg[g])
            wv_tiles[g] = t

        def qk_grant(dst_sb, pair, wtile, col0, swap_eng=None):
            """One 512-token-column projection + rope grant -> dst_sb."""
            if swap_eng is None:
                swap_eng = nc.sync
            ps = ring.tile([128, TQ], F32, tag="ring", name=f"ps_{pair}_{col0}")
            for cc in range(CC):
                nc.tensor.matmul(
                    ps[:, 0:512],
                    lhsT=wtile[:, cc, :],
                    rhs=xt_sb[:, cc, col0:col0 + 512],
                    start=(cc == 0),
                    stop=(cc == CC - 1),
                )
            xb = qkx.tile([128, 512], BF16, tag="xb", name=f"xb_{pair}_{col0}")
            nc.vector.tensor_copy(xb, ps[:, 0:512])
            u = qkx.tile([128, 512], BF16, tag="u", name=f"u_{pair}_{col0}")
            v = qkx.tile([128, 512], BF16, tag="v", name=f"v_{pair}_{col0}")
            vs = qkx.tile([128, 512], BF16, tag="vs", name=f"vs_{pair}_{col0}")
            nc.vector.tensor_mul(u, xb, ck[:, col0:col0 + 512])
            nc.vector.tensor_mul(v, xb, sk[:, col0:col0 + 512])
            for blk in range(4):
                r = blk * 32
                s = (blk ^ 1) * 32
                swap_eng.dma_start(out=vs[r:r + 32, :], in_=v[s:s + 32, :])
            nc.gpsimd.tensor_add(dst_sb[:, pair, col0:col0 + 512], u, vs)

        def v_grant(g, tt):
            """V columns for group g (8 heads), token tile tt."""
            psv = ring.tile([128, 512], F32, tag="ring", name=f"psv_{g}_{tt}")
            for cc in range(CC):
                nc.tensor.matmul(
                    psv,
                    lhsT=xt_sb[:, cc, tt * 128:(tt + 1) * 128],
                    rhs=wv_tiles[g][:, cc, :],
                    start=(cc == 0),
                    stop=(cc == CC - 1),
                )
            nc.vector.tensor_copy(
                v_sb[:, tt, 8 * g:8 * g + 8, 0:64],
                psv.rearrange("p (h d) -> p h d", h=8),
            )

        # ------------------------------------------------------- startup
        # xt arrives in 512-column pieces so the first Q grant starts as
        # soon as ~1.5MB (wq + one piece) has landed.  V grants are
        # interleaved between the QK grants to keep the PE busy during
        # the rope chains.
        load_wq(0)
        nc.sync.dma_start(out=xt_sb[:, :, 0:512], in_=xt_r[:, :, 0:512])
        load_wk(0)
        load_wv(0)
        nc.scalar.dma_start(out=ck[:, 0:512], in_=cosk[:, 0:512])
        nc.scalar.dma_start(out=sk[:, 0:512], in_=sink[:, 0:512])
        load_wq(1)
        load_wk(1)
        nc.sync.dma_start(out=xt_sb[:, :, 512:1024], in_=xt_r[:, :, 512:1024])
        nc.scalar.dma_start(out=ck[:, 512:1024], in_=cosk[:, 512:1024])
        nc.scalar.dma_start(out=sk[:, 512:1024], in_=sink[:, 512:1024])
        qk_grant(qt_sb, 0, wq_tiles[0], 0, swap_eng=nc.scalar)
        v_grant(0, 0)
        v_grant(0, 1)
        qk_grant(qt_sb, 0, wq_tiles[0], 512, swap_eng=nc.scalar)
        v_grant(0, 2)
        v_grant(0, 3)
        qk_grant(kt_sb, 0, wk_tiles[0], 0, swap_eng=nc.scalar)
        nc.sync.dma_start(out=xt_sb[:, :, 1024:1536], in_=xt_r[:, :, 1024:1536])
        nc.sync.dma_start(out=xt_sb[:, :, 1536:2048], in_=xt_r[:, :, 1536:2048])
        nc.scalar.dma_start(out=ck[:, TQ:T], in_=cosk[:, TQ:T])
        nc.scalar.dma_start(out=sk[:, TQ:T], in_=sink[:, TQ:T])

        # ------------------------------------------------- attention loop
        for p in range(NPAIR):
            if p < 6:
                load_wq(p + 2)
                load_wk(p + 2)
            if p == 1:
                # wv is single-buffered (SBUF pressure): group 1 loads
                # into group 0's slot once pair 0's V grants are done.
                load_wv(1)
            if p == 6:
                # preload W_proj so the proj phase starts immediately
                wp_r = wpt.rearrange("(cc p) e -> p cc e", p=128)
                nc.sync.dma_start(out=wp_sb[:, 0:4, :], in_=wp_r[:, 0:4, :])
                nc.sync.dma_start(out=wp_sb[:, 4:8, :], in_=wp_r[:, 4:8, :])

            def mk_v(g, tt):
                return lambda: v_grant(g, tt)

            def mk_k(q, n):
                return lambda: qk_grant(kt_sb, q, wk_tiles[q], n * 512)

            def mk_q(q, n):
                return lambda: qk_grant(qt_sb, q, wq_tiles[q], n * 512)

            if p == 0:
                # Prefill.  V(0, tt) must be emitted at iteration <= tt
                # (OT(kt=tt) runs at iteration tt+1); K0 quarter n before
                # iteration 4n.  Next-pair Q/K grants ride the back half.
                v_grant(0, 4)
                fill_at = {
                    0: [mk_v(0, 5)],
                    1: [mk_v(0, 6), mk_k(0, 1)],
                    2: [mk_v(0, 7)],
                    3: [mk_v(0, 8)],
                    4: [mk_v(0, 9)],
                    5: [mk_v(0, 10), mk_k(0, 2)],
                    6: [mk_v(0, 11)],
                    7: [mk_v(0, 12)],
                    8: [mk_v(0, 13), mk_q(1, 0)],
                    9: [mk_v(0, 14), mk_k(0, 3)],
                    10: [mk_v(0, 15), mk_q(1, 1)],
                    11: [mk_k(1, 0)],
                    12: [mk_k(1, 1)],
                    14: [mk_k(1, 2)],
                    15: [mk_k(1, 3)],
                }
            else:
                # Q(p+1) first: the next pair's first S matmul waits on
                # Q's rope add, so it must complete well before the
                # boundary.  K quarters n are needed by iteration 4n of
                # pair p+1.  V group 1 grants (needed from pair 4,
                # token-tile tt by its iteration tt) spread over pairs
                # 1-4.
                fill_at = {}
                if p < 7:
                    fill_at[0] = [mk_q(p + 1, 0)]
                    fill_at[1] = [mk_q(p + 1, 1)]
                    fill_at[2] = [mk_k(p + 1, 0)]
                    fill_at[4] = [mk_k(p + 1, 1)]
                    fill_at[10] = [mk_k(p + 1, 2)]
                    fill_at[12] = [mk_k(p + 1, 3)]
                if 1 <= p <= 4:
                    vtt = range((p - 1) * 4, (p - 1) * 4 + 4)
                    for slot, tt in zip((3, 5, 7, 9), vtt):
                        fill_at.setdefault(slot, []).append(mk_v(1, tt))

            psAB = otps.tile([128, 2, TQ], F32, tag="ot", name=f"psAB_{p}")

            def ot_mm(kt, pt0, pt1):
                # pt0/pt1 hold [head A q-chunk | head B q-chunk] for
                # q cols 0:512 / 512:1024.  A,A,B,B order reuses the
                # stationary V tile across the two q chunks.
                nc.tensor.matmul(
                    psAB[0:65, 0, 0:512],
                    lhsT=v_sb[:, kt, 2 * p, :],
                    rhs=pt0[:, 0:512],
                    start=(kt == 0),
                    stop=(kt == KT_TILES - 1),
                )
                nc.tensor.matmul(
                    psAB[0:65, 0, 512:1024],
                    lhsT=v_sb[:, kt, 2 * p, :],
                    rhs=pt1[:, 0:512],
                    start=(kt == 0),
                    stop=(kt == KT_TILES - 1),
                )
                nc.tensor.matmul(
                    psAB[0:65, 1, 0:512],
                    lhsT=v_sb[:, kt, 2 * p + 1, :],
                    rhs=pt0[:, 512:1024],
                    start=(kt == 0),
                    stop=(kt == KT_TILES - 1),
                )
                nc.tensor.matmul(
                    psAB[0:65, 1, 512:1024],
                    lhsT=v_sb[:, kt, 2 * p + 1, :],
                    rhs=pt1[:, 512:1024],
                    start=(kt == 0),
                    stop=(kt == KT_TILES - 1),
                )

            # Software-pipelined: the PE is in-order, so OT(kt) must not
            # be enqueued before S(kt+1) — it would stall the whole PE
            # queue on exp(kt).  Defer OT by one kt.
            pends = []
            for kt in range(KT_TILES):
                # Merged score tile: head A -> cols 0:512 (rows 0:64 of
                # the array), head B -> cols 512:1024 (rows 64:128).
                # The two matmuls hit different PSUM banks and row
                # groups -> they can run concurrently.
                st0 = ring.tile([128, TQ], F32, tag="ring",
                                name=f"st0_{p}_{kt}")
                st1 = ring.tile([128, TQ], F32, tag="ring",
                                name=f"st1_{p}_{kt}")
                nc.tensor.matmul(
                    st0[:, 0:512],
                    lhsT=kt_sb[0:64, p, kt * 128:(kt + 1) * 128],
                    rhs=qt_sb[0:64, p, 0:512],
                    start=True, stop=True, tile_position=(0, 0),
                )
                nc.tensor.matmul(
                    st0[:, 512:1024],
                    lhsT=kt_sb[64:128, p, kt * 128:(kt + 1) * 128],
                    rhs=qt_sb[64:128, p, 0:512],
                    start=True, stop=True, tile_position=(64, 0),
                )
                pt0 = ptp.tile([128, TQ], BF16, tag="pt",
                               name=f"pt0_{p}_{kt}")
                nc.scalar.activation(
                    out=pt0, in_=st0,
                    func=mybir.ActivationFunctionType.Exp, scale=SCALE,
                )
                nc.tensor.matmul(
                    st1[:, 0:512],
                    lhsT=kt_sb[0:64, p, kt * 128:(kt + 1) * 128],
                    rhs=qt_sb[0:64, p, 512:1024],
                    start=True, stop=True, tile_position=(0, 0),
                )
                nc.tensor.matmul(
                    st1[:, 512:1024],
                    lhsT=kt_sb[64:128, p, kt * 128:(kt + 1) * 128],
                    rhs=qt_sb[64:128, p, 512:1024],
                    start=True, stop=True, tile_position=(64, 0),
                )
                pt1 = ptp.tile([128, TQ], BF16, tag="pt",
                               name=f"pt1_{p}_{kt}")
                nc.scalar.activation(
                    out=pt1, in_=st1,
                    func=mybir.ActivationFunctionType.Exp, scale=SCALE,
                )
                if len(pends) == 2:
                    ot_mm(*pends.pop(0))
                pends.append((kt, pt0, pt1))
                for f in fill_at.get(kt, ()):
                    f()
            for e in pends:
                ot_mm(*e)

            # --------------------------------------------- pair epilogue
            # Value rows -> bf16 SBUF (frees most of psAB); the fp32
            # denominator row DMAs to DRAM, broadcast-reads back onto 64
            # partitions, and the reciprocal+normalize run on the DVE.
            # ACT stays exp-only all kernel.
            attU = eps.tile([65, 2, TQ], BF16, tag="attU", bufs=1,
                            name=f"attU_{p}")
            nc.vector.tensor_copy(attU, psAB[0:65, :, :])
            nc.sync.dma_start(out=rs_dram[p], in_=attU[64:65, :, :])
            # Reciprocal on the COMPACT denominator: round-trip through
            # DRAM to reshape [1, 2048] -> [128, 16], so the (slow) DVE
            # reciprocal touches only 16 elems/lane instead of 2048.
            den128 = eps.tile([128, 16], BF16, tag="den128", bufs=1,
                              name=f"den128_{p}")
            nc.sync.dma_start(out=den128, in_=rs_dram[p])
            denf = eps.tile([128, 16], F32, tag="denf", bufs=1,
                            name=f"denf_{p}")
            nc.vector.tensor_copy(denf, den128)
            recf = eps.tile([128, 16], F32, tag="recf", bufs=1,
                            name=f"recf_{p}")
            nc.vector.reciprocal(recf, denf)
            rec128 = eps.tile([128, 16], BF16, tag="rec128", bufs=1,
                              name=f"rec128_{p}")
            nc.vector.tensor_copy(rec128, recf)
            nc.sync.dma_start(out=rc_dram[p], in_=rec128)
            rbc = eps.tile([64, 2, TQ], BF16, tag="rbc", bufs=1,
                           name=f"rbc_{p}")
            nc.sync.dma_start(
                out=rbc[:, 0, :],
                in_=rc_dram[p, 0:1, :].broadcast_to([64, TQ]))
            nc.sync.dma_start(
                out=rbc[:, 1, :],
                in_=rc_dram[p, 1:2, :].broadcast_to([64, TQ]))
            attB = eps.tile([64, TQ], BF16, tag="attB", bufs=1,
                            name=f"attB_{p}")
            nc.vector.tensor_mul(att_sb[0:64, p, :], attU[0:64, 0, :],
                                 rbc[:, 0, :])
            nc.vector.tensor_mul(attB, attU[0:64, 1, :], rbc[:, 1, :])
            nc.sync.dma_start(out=att_sb[64:128, p, :], in_=attB)


def _phase_proj(nc, tc, wp_sb, att_sb, out_ext):
    """out = attT^T @ WpT, per 128-token tile.  Pair 7's contribution is
    deferred by two token tiles so its late-normalized att never blocks
    the in-order PE queue."""
    with tc.tile_pool(name="pph", bufs=2) as pph, \
         tc.tile_pool(name="pps", bufs=3, space="PSUM") as pps:
        NTT = TQ // 128

        def head_mm(ps, tt):
            for p in range(NPAIR - 1):
                for nch in range(2):
                    nc.tensor.matmul(
                        ps[:, nch * 512:(nch + 1) * 512],
                        lhsT=att_sb[:, p, tt * 128:(tt + 1) * 128],
                        rhs=wp_sb[:, p, nch * 512:(nch + 1) * 512],
                        start=(p == 0),
                        stop=False,
                    )

        def tail_mm(ps, tt):
            for nch in range(2):
                nc.tensor.matmul(
                    ps[:, nch * 512:(nch + 1) * 512],
                    lhsT=att_sb[:, NPAIR - 1, tt * 128:(tt + 1) * 128],
                    rhs=wp_sb[:, NPAIR - 1, nch * 512:(nch + 1) * 512],
                    start=False,
                    stop=True,
                )
            o = pph.tile([128, C], F32, tag="o", name=f"o_{tt}")
            if tt % 2 == 0:
                nc.vector.tensor_copy(o, ps)
            else:
                nc.scalar.activation(
                    out=o, in_=ps, func=mybir.ActivationFunctionType.Copy
                )
            nc.sync.dma_start(out=out_ext[tt * 128:(tt + 1) * 128, :], in_=o)

        pend = []
        for tt in range(NTT):
            ps = pps.tile([128, C], F32, tag="ps", name=f"ps_{tt}")
            head_mm(ps, tt)
            pend.append((ps, tt))
            if len(pend) > 2:
                tail_mm(*pend.pop(0))
        for e in pend:
            tail_mm(*e)


_NC_CACHE = None


def _get_nc():
    global _NC_CACHE
    if _NC_CACHE is None:
        _NC_CACHE = _build_nc()
    return _NC_CACHE


# ---------------------------------------------------------------------------
# Host wrapper
# ---------------------------------------------------------------------------

def kernel(x, W_qkv, W_proj, cos, sin, mask):
    bf = ml_dtypes.bfloat16
    x = np.asarray(x, dtype=np.float32)
    W_qkv = np.asarray(W_qkv, dtype=np.float32)
    W_proj = np.asarray(W_proj, dtype=np.float32)
    cos = np.asarray(cos, dtype=np.float32)
    sin = np.asarray(sin, dtype=np.float32)

    # Permute q/k head dims: interleaved (x1,x2 pairs) -> halves [x1; x2].
    perm = np.concatenate([np.arange(0, HD, 2), np.arange(1, HD, 2)])
    Wq = W_qkv[0:C].reshape(H, HD, C)[:, perm, :].reshape(C, C)
    Wk = W_qkv[C:2 * C].reshape(H, HD, C)[:, perm, :].reshape(C, C)
    Wv = W_qkv[2 * C:3 * C]

    # per-pair tiled layouts: [NPAIR, 128 c-part, CC, 128 d]
    wqt = np.ascontiguousarray(
        Wq.T.astype(bf).reshape(CC, 128, NPAIR, 128).transpose(2, 1, 0, 3)
    )
    wkt = np.ascontiguousarray(
        Wk.T.astype(bf).reshape(CC, 128, NPAIR, 128).transpose(2, 1, 0, 3)
    )
    # V weights in 4-pair (512 v-dim) group slabs: [NVG, 128 c-part, CC, 512]
    wvg = np.ascontiguousarray(
        Wv.T.astype(bf).reshape(CC, 128, NVG, 512).transpose(2, 1, 0, 3)
    )
    wpt = np.ascontiguousarray(W_proj.T.astype(bf))

    # RoPE tables in transposed/replicated layout:
    #   cosr[r, t] = cos[t, r % 32]
    #   sinB[r, t] = +sin[t, r%32] for (r%64)<32 else -sin[t, r%32]
    cosT = cos.T
    sinT = sin.T
    cosr = np.ascontiguousarray(np.tile(cosT, (4, 1)).astype(bf))
    sinB = np.ascontiguousarray(
        np.tile(np.concatenate([sinT, -sinT], axis=0), (2, 1)).astype(bf)
    )

    in_maps = []
    for c in range(NCORES):
        b, hf = divmod(c, 2)
        qs = hf * TQ
        # token order per core: own q half first, partner half second
        # (attention is permutation-invariant over k tokens as long as
        # KT / V / rope tables all use the same order)
        ordr = np.concatenate(
            [np.arange(qs, qs + TQ), np.arange((TQ + qs) % T, (TQ + qs) % T + TQ)]
        )
        xtb = np.ascontiguousarray(x[b].T.astype(bf)[:, ordr])
        in_maps.append(
            {
                "xt": xtb,
                "wqt": wqt,
                "wkt": wkt,
                "wvg": wvg,
                "wpt": wpt,
                "cosk": np.ascontiguousarray(cosr[:, ordr]),
                "sink": np.ascontiguousarray(sinB[:, ordr]),
            }
        )

    nc = _get_nc()
    trace = bool(int(os.environ.get("BASSK_TRACE", "0")))
    res = bass_utils.run_bass_kernel_spmd(
        nc, in_maps, core_ids=list(range(NCORES)), trace=trace
    )
    if trace:
        kernel.last_exec_time_ns = res.exec_time_ns
        kernel.last_profile = res

    out = np.empty((B, T, C), dtype=np.float32)
    for c in range(NCORES):
        b, hf = divmod(c, 2)
        qs = hf * TQ
        out[b, qs:qs + TQ, :] = res.results[c]["out"]
    return out


# revision 13
# speedup vs baseline: 1.3392x; 1.1076x over previous
"""Distributed Trainium2 Bass kernel for nn_Attention_62766652063769 (v3).

Reference computation (B=4, T=2048, C=1024, H=16, HD=64):
    qkv = x @ W_qkv^T ; split into q, k, v heads
    q, k <- RoPE(q), RoPE(k)   (interleaved-pair rotation)
    attn = softmax(q k^T / sqrt(HD))   (mask is all-ones -> no masking)
    out  = (attn @ v) @ W_proj^T

Sharding: 8 cores; core c owns batch b = c//2 and query-token half c%2
(1024 q tokens).  K/V for the full 2048-token batch are computed
redundantly by both cores of a pair - zero inter-core communication.

v3 design (vs v2 at ~570us):
  - Merged score tiles: one PSUM tile [128, 1024] per (kt, q-half)
    holds BOTH heads side by side (head A cols 0:512 via
    tile_position (0,0), head B cols 512:1024 via (64,0)).  The two
    64-contraction matmuls target different PSUM banks and different
    row groups -> hardware row-tiling concurrency; and the exp for a
    tile can start after only 2 matmuls.  Ring recycle waits also
    clear a full exp earlier than the old stA/stB split.
  - Softmax denominator reciprocal moved off ACT: DMA the fp32 den
    row to DRAM, broadcast-read it to 64 partitions, DVE
    reciprocal_approx_fast + muls.  ACT now runs ONLY Exp; the
    pair-7 epilogue chain that gated the proj tail shrinks ~7us.
  - Fillers reordered: Q(p+1)/K(p+1,q0) grants are emitted FIRST in
    each pair (the old order put them last, stalling the next pair's
    first S matmul ~3.2us on the late rope add).
  - Weights prefetch two pairs ahead so iteration-0 fillers never
    wait on their weight DMA.
  - V projections merged into 512-wide matmuls (8 heads per grant).
  - OT matmuls ordered A,A,B,B for stationary reuse.
  - Prologue: xt loaded in 512-column pieces so the first Q grant
    starts ~3us earlier; V grants interleaved between startup QK
    grants to cover the rope-chain latency.
  - Proj tail: 3 PSUM bufs + depth-2 tail deferral.
"""

import os
import re
import sys
import types

if "/opt/trn_rl_repo" not in sys.path:
    sys.path.insert(0, "/opt/trn_rl_repo")

import ml_dtypes
import numpy as np

import bass_rust
import concourse.bass as bass
import concourse.mybir as mybir
from concourse import bass_utils
from concourse.tile import TileContext, ScopedClock

# ---------------------------------------------------------------------------
# Environment patches (same as v1/v2)
# ---------------------------------------------------------------------------

def _patched_drain_and_barrier(self, tick_clock, wait_clock):
    """The walrus build in this container encodes at most one sync-wait per
    instruction; Tile's tail drain carries one wait per live semaphore.
    Emit single-wait NOPs on SP instead, then an unguarded drain."""
    gc = tick_clock.global_clock
    ticks = [int(x) for x in re.findall(r"\d+", repr(gc))]
    for i, t in enumerate(ticks):
        if t <= 0:
            continue
        l = [0] * len(ticks)
        l[i] = t
        nop = self.nc.sync.nop(nofuse=True)
        wait_clock.add_sem_waits(nop.ins, ScopedClock({None: bass_rust.VectorClock(l)}))
    self.nc.sync.drain()
    self.nc.all_engine_barrier()
    assert self.sems is not None
    popped = self.nc._tile_sem_poison_stack.pop()
    assert popped is self._sem_poison
    self.nc.clear_and_free_semaphores(list(self.sems.allocated().values()))
    self.nc.all_engine_barrier()


TileContext._drain_and_barrier = _patched_drain_and_barrier


def _split_multi_waits(nc):
    """Move extra sync-waits onto single-wait NOPs inserted just before the
    owning instruction on the same (in-order) engine."""
    for func in nc.m.functions:
        for bb in func.blocks:
            insts = bb.instructions
            if not any(
                i.sync_info is not None
                and i.sync_info.on_wait
                and len(i.sync_info.on_wait) > 1
                for i in insts
            ):
                continue
            new = []
            for inst in insts:
                si = inst.sync_info
                if si is not None and si.on_wait and len(si.on_wait) > 1:
                    waits = list(si.on_wait)
                    for w in waits[:-1]:
                        nop = mybir.InstNoOp(
                            name=nc.get_next_instruction_name(),
                            engine=inst.engine,
                            bass_nofuse=True,
                            sync_info=mybir.SyncInfo(on_wait=[w], on_update=[]),
                        )
                        nc.register_instruction(nop)
                        new.append(nop)
                    inst.sync_info = mybir.SyncInfo(
                        on_wait=[waits[-1]], on_update=list(si.on_update)
                    )
                new.append(inst)
            bb.instructions = new


def _install_ntff_hook():
    """Recreate antenv.axon_hooks (absent in this image) so
    run_bass_kernel_spmd(trace=True) can profile through libaxon_pjrt."""
    if "antenv.axon_hooks" in sys.modules:
        return
    import contextlib
    import ctypes

    mod = types.ModuleType("antenv.axon_hooks")
    _state = {"hook": None}

    def set_axon_ntff_profile_hook(hook):
        _state["hook"] = hook

    def get_axon_ntff_profile_hook():
        return _state["hook"]

    def _ntff_profile_via_ctypes(so_path):
        lib = ctypes.CDLL(so_path)
        if not hasattr(lib, "axon_start_nrt_profile"):
            return None
        lib.axon_start_nrt_profile.argtypes = [
            ctypes.POINTER(ctypes.c_int64),
            ctypes.c_size_t,
        ]
        lib.axon_start_nrt_profile.restype = ctypes.c_int64
        lib.axon_stop_nrt_profile.argtypes = [ctypes.c_char_p]
        lib.axon_stop_nrt_profile.restype = ctypes.c_int64

        @contextlib.contextmanager
        def _hook(output_dir, device_ids):
            import jax

            jax.devices()
            if device_ids:
                ids = (ctypes.c_int64 * len(device_ids))(*device_ids)
                rc = lib.axon_start_nrt_profile(ids, len(device_ids))
            else:
                rc = lib.axon_start_nrt_profile(None, 0)
            if rc != 0:
                raise RuntimeError(f"axon_start_nrt_profile rc={rc}")
            try:
                yield
            finally:
                n = lib.axon_stop_nrt_profile(str(output_dir).encode())
                if n < 0:
                    raise RuntimeError(f"axon_stop_nrt_profile rc={n}")
                print(f"profile: {n} file(s) in {output_dir}", file=sys.stderr)

        return _hook

    mod.set_axon_ntff_profile_hook = set_axon_ntff_profile_hook
    mod.get_axon_ntff_profile_hook = get_axon_ntff_profile_hook
    try:
        set_axon_ntff_profile_hook(
            _ntff_profile_via_ctypes("/opt/axon/libaxon_pjrt.so")
        )
    except Exception:
        pass
    sys.modules["antenv.axon_hooks"] = mod
    try:
        import antenv

        antenv.axon_hooks = mod
    except ImportError:
        pass


_install_ntff_hook()

# ---------------------------------------------------------------------------
# Problem constants
# ---------------------------------------------------------------------------

B, T, C = 4, 2048, 1024
H, HD = 16, 64
NCORES = 8
TQ = T // 2          # q tokens per core
NPAIR = H // 2       # head pairs (=8); pair p holds heads 2p, 2p+1
NVG = 2              # V groups of 4 pairs (8 heads, 512 v dims)
KT_TILES = T // 128  # 16
SCALE = 1.0 / np.sqrt(HD)

F32 = mybir.dt.float32
BF16 = mybir.dt.bfloat16

CC = C // 128  # 8 contraction chunks


# ---------------------------------------------------------------------------
# Device program
# ---------------------------------------------------------------------------

def _build_nc():
    nc = bass.Bass(trn_type="TRN2", target_bir_lowering=False, debug=False)

    xt = nc.declare_dram_parameter("xt", [C, T], BF16, isOutput=False)
    wqt = nc.declare_dram_parameter("wqt", [NPAIR, 128, CC, 128], BF16,
                                    isOutput=False)
    wkt = nc.declare_dram_parameter("wkt", [NPAIR, 128, CC, 128], BF16,
                                    isOutput=False)
    wvg = nc.declare_dram_parameter("wvg", [NVG, 128, CC, 512], BF16,
                                    isOutput=False)
    wpt = nc.declare_dram_parameter("wpt", [C, C], BF16, isOutput=False)
    cosk = nc.declare_dram_parameter("cosk", [128, T], BF16, isOutput=False)
    sink = nc.declare_dram_parameter("sink", [128, T], BF16, isOutput=False)
    out_ext = nc.declare_dram_parameter("out", [TQ, C], F32, isOutput=True)

    rs_dram = nc.dram_tensor("rs_scratch", [NPAIR, 2, TQ], BF16)
    rc_dram = nc.dram_tensor("rc_scratch", [NPAIR, 2, TQ], BF16)

    with TileContext(nc) as tc:
        with tc.tile_pool(name="persist", bufs=1) as persist:
            qt_sb = persist.tile([128, NPAIR, TQ], BF16, tag="qt")
            att_sb = persist.tile([128, NPAIR, TQ], BF16, tag="att")
            kt_sb = persist.tile([128, NPAIR, T], BF16, tag="kt")
            v_sb = persist.tile([128, KT_TILES, H, 65], BF16, tag="v")
            ck = persist.tile([128, T], BF16, tag="ck")
            sk = persist.tile([128, T], BF16, tag="sk")

            with tc.tile_pool(name="xtpool", bufs=1) as xtpool, \
                 tc.tile_pool(name="pw", bufs=1) as pw:
                xt_sb = xtpool.tile([128, CC, T], BF16, tag="xt")
                xt_r = xt.rearrange("(cc p) t -> p cc t", p=128)
                nc.vector.memset(v_sb[:, :, :, 64:65], 1.0)

                wp_sb = pw.tile([128, CC, C], BF16)

                _attention(nc, tc, xt_sb, qt_sb, kt_sb, v_sb, att_sb,
                           ck, sk, wqt, wkt, wvg, rs_dram, rc_dram,
                           wp_sb, wpt, xt_r, cosk, sink)

                _phase_proj(nc, tc, wp_sb, att_sb, out_ext)

    _split_multi_waits(nc)
    return nc


def _attention(nc, tc, xt_sb, qt_sb, kt_sb, v_sb, att_sb, ck, sk,
               wqt, wkt, wvg, rs_dram, rc_dram, wp_sb, wpt, xt_r, cosk,
               sink):
    with tc.tile_pool(name="ring", bufs=2, space="PSUM") as ring, \
         tc.tile_pool(name="otps", bufs=1, space="PSUM") as otps, \
         tc.tile_pool(name="wts", bufs=2) as wts, \
         tc.tile_pool(name="qkx", bufs=2) as qkx, \
         tc.tile_pool(name="ptp", bufs=6) as ptp, \
         tc.tile_pool(name="eps", bufs=1) as eps:

        wq_tiles = {}
        wk_tiles = {}
        wv_tiles = {}

        def load_wq(p, eng=None):
            t = wts.tile([128, CC, 128], BF16, tag="wq")
            (eng or nc.gpsimd).dma_start(out=t, in_=wqt[p])
            wq_tiles[p] = t

        def load_wk(p, eng=None):
            t = wts.tile([128, CC, 128], BF16, tag="wk")
            (eng or nc.gpsimd).dma_start(out=t, in_=wkt[p])
            wk_tiles[p] = t

        def load_wv(g, eng=None):
            t = wts.tile([128, CC, 512], BF16, tag="wv", bufs=1)
            (eng or nc.gpsimd).dma_start(out=t, in_=wvg[g])
            wv_tiles[g] = t

        def qk_grant(dst_sb, pair, wtile, col0, swap_eng=None):
            """One 512-token-column projection + rope grant -> dst_sb."""
            if swap_eng is None:
                swap_eng = nc.sync
            ps = ring.tile([128, TQ], F32, tag="ring", name=f"ps_{pair}_{col0}")
            for cc in range(CC):
                nc.tensor.matmul(
                    ps[:, 0:512],
                    lhsT=wtile[:, cc, :],
                    rhs=xt_sb[:, cc, col0:col0 + 512],
                    start=(cc == 0),
                    stop=(cc == CC - 1),
                )
            xb = qkx.tile([128, 512], BF16, tag="xb", name=f"xb_{pair}_{col0}")
            nc.vector.tensor_copy(xb, ps[:, 0:512])
            u = qkx.tile([128, 512], BF16, tag="u", name=f"u_{pair}_{col0}")
            v = qkx.tile([128, 512], BF16, tag="v", name=f"v_{pair}_{col0}")
            vs = qkx.tile([128, 512], BF16, tag="vs", name=f"vs_{pair}_{col0}")
            nc.vector.tensor_mul(u, xb, ck[:, col0:col0 + 512])
            nc.vector.tensor_mul(v, xb, sk[:, col0:col0 + 512])
            for blk in range(4):
                r = blk * 32
                s = (blk ^ 1) * 32
                swap_eng.dma_start(out=vs[r:r + 32, :], in_=v[s:s + 32, :])
            nc.gpsimd.tensor_add(dst_sb[:, pair, col0:col0 + 512], u, vs)

        def v_grant(g, tt):
            """V columns for group g (8 heads), token tile tt."""
            psv = ring.tile([128, 512], F32, tag="ring", name=f"psv_{g}_{tt}")
            for cc in range(CC):
                nc.tensor.matmul(
                    psv,
                    lhsT=xt_sb[:, cc, tt * 128:(tt + 1) * 128],
                    rhs=wv_tiles[g][:, cc, :],
                    start=(cc == 0),
                    stop=(cc == CC - 1),
                )
            nc.vector.tensor_copy(
                v_sb[:, tt, 8 * g:8 * g + 8, 0:64],
                psv.rearrange("p (h d) -> p h d", h=8),
            )

        # ------------------------------------------------------- startup
        # xt arrives in 512-column pieces so the first Q grant starts as
        # soon as ~1.5MB (wq + one piece) has landed.  V grants are
        # interleaved between the QK grants to keep the PE busy during
        # the rope chains.
        load_wq(0, eng=nc.scalar)
        nc.sync.dma_start(out=xt_sb[:, :, 0:512], in_=xt_r[:, :, 0:512])
        load_wk(0, eng=nc.scalar)
        nc.scalar.dma_start(out=ck[:, 0:512], in_=cosk[:, 0:512])
        nc.scalar.dma_start(out=sk[:, 0:512], in_=sink[:, 0:512])
        nc.sync.dma_start(out=xt_sb[:, :, 512:1024], in_=xt_r[:, :, 512:1024])
        load_wv(0, eng=nc.gpsimd)
        load_wq(1)
        load_wk(1)
        nc.scalar.dma_start(out=ck[:, 512:1024], in_=cosk[:, 512:1024])
        nc.scalar.dma_start(out=sk[:, 512:1024], in_=sink[:, 512:1024])
        qk_grant(qt_sb, 0, wq_tiles[0], 0, swap_eng=nc.scalar)
        qk_grant(kt_sb, 0, wk_tiles[0], 0, swap_eng=nc.scalar)
        qk_grant(qt_sb, 0, wq_tiles[0], 512, swap_eng=nc.scalar)
        v_grant(0, 0)
        v_grant(0, 1)
        v_grant(0, 2)
        v_grant(0, 3)
        nc.sync.dma_start(out=xt_sb[:, :, 1024:1536], in_=xt_r[:, :, 1024:1536])
        nc.sync.dma_start(out=xt_sb[:, :, 1536:2048], in_=xt_r[:, :, 1536:2048])
        nc.scalar.dma_start(out=ck[:, TQ:T], in_=cosk[:, TQ:T])
        nc.scalar.dma_start(out=sk[:, TQ:T], in_=sink[:, TQ:T])

        # ------------------------------------------------- attention loop
        for p in range(NPAIR):
            if p < 6:
                load_wq(p + 2)
                load_wk(p + 2)
            if p == 1:
                # wv is single-buffered (SBUF pressure): group 1 loads
                # into group 0's slot once pair 0's V grants are done.
                load_wv(1)
            if p == 6:
                # preload W_proj so the proj phase starts immediately
                wp_r = wpt.rearrange("(cc p) e -> p cc e", p=128)
                nc.sync.dma_start(out=wp_sb[:, 0:4, :], in_=wp_r[:, 0:4, :])
                nc.sync.dma_start(out=wp_sb[:, 4:8, :], in_=wp_r[:, 4:8, :])

            def mk_v(g, tt):
                return lambda: v_grant(g, tt)

            def mk_k(q, n):
                return lambda: qk_grant(kt_sb, q, wk_tiles[q], n * 512)

            def mk_q(q, n):
                return lambda: qk_grant(qt_sb, q, wq_tiles[q], n * 512)

            if p == 0:
                # Prefill.  V(0, tt) must be emitted at iteration <= tt
                # (OT(kt=tt) runs at iteration tt+1); K0 quarter n before
                # iteration 4n.  Next-pair Q/K grants ride the back half.
                v_grant(0, 4)
                fill_at = {
                    0: [mk_v(0, 5)],
                    1: [mk_v(0, 6), mk_k(0, 1)],
                    2: [mk_v(0, 7)],
                    3: [mk_v(0, 8)],
                    4: [mk_v(0, 9)],
                    5: [mk_v(0, 10), mk_k(0, 2)],
                    6: [mk_v(0, 11)],
                    7: [mk_v(0, 12)],
                    8: [mk_v(0, 13), mk_q(1, 0)],
                    9: [mk_v(0, 14), mk_k(0, 3)],
                    10: [mk_v(0, 15), mk_q(1, 1)],
                    11: [mk_k(1, 0)],
                    12: [mk_k(1, 1)],
                    14: [mk_k(1, 2)],
                    15: [mk_k(1, 3)],
                }
            else:
                # Q(p+1) first: the next pair's first S matmul waits on
                # Q's rope add, so it must complete well before the
                # boundary.  K quarters n are needed by iteration 4n of
                # pair p+1.  V group 1 grants (needed from pair 4,
                # token-tile tt by its iteration tt) spread over pairs
                # 1-4.
                fill_at = {}
                if p < 7:
                    fill_at[0] = [mk_q(p + 1, 0)]
                    fill_at[1] = [mk_q(p + 1, 1)]
                    fill_at[2] = [mk_k(p + 1, 0)]
                    fill_at[4] = [mk_k(p + 1, 1)]
                    fill_at[10] = [mk_k(p + 1, 2)]
                    fill_at[12] = [mk_k(p + 1, 3)]
                if 1 <= p <= 4:
                    vtt = range((p - 1) * 4, (p - 1) * 4 + 4)
                    for slot, tt in zip((3, 5, 7, 9), vtt):
                        fill_at.setdefault(slot, []).append(mk_v(1, tt))

            psAB = otps.tile([128, 2, TQ], F32, tag="ot", name=f"psAB_{p}")

            def ot_mm_A(kt, pt0, pt1):
                # head A: stationary V reused across the two q chunks.
                nc.tensor.matmul(
                    psAB[0:65, 0, 0:512],
                    lhsT=v_sb[:, kt, 2 * p, :],
                    rhs=pt0[:, 0:512],
                    start=(kt == 0),
                    stop=(kt == KT_TILES - 1),
                )
                nc.tensor.matmul(
                    psAB[0:65, 0, 512:1024],
                    lhsT=v_sb[:, kt, 2 * p, :],
                    rhs=pt1[:, 0:512],
                    start=(kt == 0),
                    stop=(kt == KT_TILES - 1),
                )

            def ot_mm_B(kt, pt0, pt1):
                nc.tensor.matmul(
                    psAB[0:65, 1, 0:512],
                    lhsT=v_sb[:, kt, 2 * p + 1, :],
                    rhs=pt0[:, 512:1024],
                    start=(kt == 0),
                    stop=(kt == KT_TILES - 1),
                )
                nc.tensor.matmul(
                    psAB[0:65, 1, 512:1024],
                    lhsT=v_sb[:, kt, 2 * p + 1, :],
                    rhs=pt1[:, 512:1024],
                    start=(kt == 0),
                    stop=(kt == KT_TILES - 1),
                )

            # Software-pipelined: the PE is in-order, so OT(kt) must not
            # be enqueued before S(kt+1) — it would stall the whole PE
            # queue on exp(kt).  Defer OT by one kt.
            pends = []
            for kt in range(KT_TILES):
                # Merged score tile: head A -> cols 0:512 (rows 0:64 of
                # the array), head B -> cols 512:1024 (rows 64:128).
                # The two matmuls hit different PSUM banks and row
                # groups -> they can run concurrently.
                st0 = ring.tile([128, TQ], F32, tag="ring",
                                name=f"st0_{p}_{kt}")
                st1 = ring.tile([128, TQ], F32, tag="ring",
                                name=f"st1_{p}_{kt}")
                nc.tensor.matmul(
                    st0[:, 0:512],
                    lhsT=kt_sb[0:64, p, kt * 128:(kt + 1) * 128],
                    rhs=qt_sb[0:64, p, 0:512],
                    start=True, stop=True, tile_position=(0, 0),
                )
                nc.tensor.matmul(
                    st0[:, 512:1024],
                    lhsT=kt_sb[64:128, p, kt * 128:(kt + 1) * 128],
                    rhs=qt_sb[64:128, p, 0:512],
                    start=True, stop=True, tile_position=(64, 0),
                )
                pt0 = ptp.tile([128, TQ], BF16, tag="pt",
                               name=f"pt0_{p}_{kt}")
                nc.scalar.activation(
                    out=pt0, in_=st0,
                    func=mybir.ActivationFunctionType.Exp, scale=SCALE,
                )
                if len(pends) == 2:
                    ot_mm_A(*pends[0])
                nc.tensor.matmul(
                    st1[:, 0:512],
                    lhsT=kt_sb[0:64, p, kt * 128:(kt + 1) * 128],
                    rhs=qt_sb[0:64, p, 512:1024],
                    start=True, stop=True, tile_position=(0, 0),
                )
                nc.tensor.matmul(
                    st1[:, 512:1024],
                    lhsT=kt_sb[64:128, p, kt * 128:(kt + 1) * 128],
                    rhs=qt_sb[64:128, p, 512:1024],
                    start=True, stop=True, tile_position=(64, 0),
                )
                pt1 = ptp.tile([128, TQ], BF16, tag="pt",
                               name=f"pt1_{p}_{kt}")
                nc.scalar.activation(
                    out=pt1, in_=st1,
                    func=mybir.ActivationFunctionType.Exp, scale=SCALE,
                )
                if len(pends) == 2:
                    ot_mm_B(*pends.pop(0))
                pends.append((kt, pt0, pt1))
                for f in fill_at.get(kt, ()):
                    f()
            for e in pends:
                ot_mm_A(*e)
                ot_mm_B(*e)

            # --------------------------------------------- pair epilogue
            # Value rows -> bf16 SBUF (frees most of psAB); the fp32
            # denominator row DMAs to DRAM, broadcast-reads back onto 64
            # partitions, and the reciprocal+normalize run on the DVE.
            # ACT stays exp-only all kernel.
            attU = eps.tile([65, 2, TQ], BF16, tag="attU", bufs=1,
                            name=f"attU_{p}")
            nc.vector.tensor_copy(attU, psAB[0:65, :, :])
            nc.sync.dma_start(out=rs_dram[p], in_=attU[64:65, :, :])
            # Reciprocal on the COMPACT denominator: round-trip through
            # DRAM to reshape [1, 2048] -> [128, 16], so the (slow) DVE
            # reciprocal touches only 16 elems/lane instead of 2048.
            den128 = eps.tile([128, 16], BF16, tag="den128", bufs=1,
                              name=f"den128_{p}")
            nc.sync.dma_start(out=den128, in_=rs_dram[p])
            denf = eps.tile([128, 16], F32, tag="denf", bufs=1,
                            name=f"denf_{p}")
            nc.vector.tensor_copy(denf, den128)
            recf = eps.tile([128, 16], F32, tag="recf", bufs=1,
                            name=f"recf_{p}")
            nc.vector.reciprocal(recf, denf)
            rec128 = eps.tile([128, 16], BF16, tag="rec128", bufs=1,
                              name=f"rec128_{p}")
            nc.vector.tensor_copy(rec128, recf)
            nc.sync.dma_start(out=rc_dram[p], in_=rec128)
            rbc = eps.tile([64, 2, TQ], BF16, tag="rbc", bufs=1,
                           name=f"rbc_{p}")
            nc.sync.dma_start(
                out=rbc[:, 0, :],
                in_=rc_dram[p, 0:1, :].broadcast_to([64, TQ]))
            nc.sync.dma_start(
                out=rbc[:, 1, :],
                in_=rc_dram[p, 1:2, :].broadcast_to([64, TQ]))
            attB = eps.tile([64, TQ], BF16, tag="attB", bufs=1,
                            name=f"attB_{p}")
            nc.vector.tensor_mul(att_sb[0:64, p, :], attU[0:64, 0, :],
                                 rbc[:, 0, :])
            nc.vector.tensor_mul(attB, attU[0:64, 1, :], rbc[:, 1, :])
            nc.sync.dma_start(out=att_sb[64:128, p, :], in_=attB)


def _phase_proj(nc, tc, wp_sb, att_sb, out_ext):
    """out = attT^T @ WpT, per 128-token tile.  Pair 7's contribution is
    deferred by two token tiles so its late-normalized att never blocks
    the in-order PE queue."""
    with tc.tile_pool(name="pph", bufs=2) as pph, \
         tc.tile_pool(name="pps", bufs=4, space="PSUM") as pps:
        NTT = TQ // 128

        def head_mm(ps, tt):
            for p in range(NPAIR - 1):
                for nch in range(2):
                    nc.tensor.matmul(
                        ps[:, nch * 512:(nch + 1) * 512],
                        lhsT=att_sb[:, p, tt * 128:(tt + 1) * 128],
                        rhs=wp_sb[:, p, nch * 512:(nch + 1) * 512],
                        start=(p == 0),
                        stop=False,
                    )

        def tail_mm(ps, tt):
            for nch in range(2):
                nc.tensor.matmul(
                    ps[:, nch * 512:(nch + 1) * 512],
                    lhsT=att_sb[:, NPAIR - 1, tt * 128:(tt + 1) * 128],
                    rhs=wp_sb[:, NPAIR - 1, nch * 512:(nch + 1) * 512],
                    start=False,
                    stop=True,
                )
            o = pph.tile([128, C], F32, tag="o", name=f"o_{tt}")
            if tt % 2 == 0:
                nc.vector.tensor_copy(o, ps)
            else:
                nc.scalar.activation(
                    out=o, in_=ps, func=mybir.ActivationFunctionType.Copy
                )
            eng = nc.sync if tt % 2 == 0 else nc.scalar
            eng.dma_start(out=out_ext[tt * 128:(tt + 1) * 128, :], in_=o)

        pend = []
        for tt in range(NTT):
            ps = pps.tile([128, C], F32, tag="ps", name=f"ps_{tt}")
            head_mm(ps, tt)
            pend.append((ps, tt))
            if len(pend) > 3:
                tail_mm(*pend.pop(0))
        for e in pend:
            tail_mm(*e)


_NC_CACHE = None


def _get_nc():
    global _NC_CACHE
    if _NC_CACHE is None:
        _NC_CACHE = _build_nc()
    return _NC_CACHE


# ---------------------------------------------------------------------------
# Host wrapper
# ---------------------------------------------------------------------------

def kernel(x, W_qkv, W_proj, cos, sin, mask):
    bf = ml_dtypes.bfloat16
    x = np.asarray(x, dtype=np.float32)
    W_qkv = np.asarray(W_qkv, dtype=np.float32)
    W_proj = np.asarray(W_proj, dtype=np.float32)
    cos = np.asarray(cos, dtype=np.float32)
    sin = np.asarray(sin, dtype=np.float32)

    # Permute q/k head dims: interleaved (x1,x2 pairs) -> halves [x1; x2].
    perm = np.concatenate([np.arange(0, HD, 2), np.arange(1, HD, 2)])
    Wq = W_qkv[0:C].reshape(H, HD, C)[:, perm, :].reshape(C, C)
    Wk = W_qkv[C:2 * C].reshape(H, HD, C)[:, perm, :].reshape(C, C)
    Wv = W_qkv[2 * C:3 * C]

    # per-pair tiled layouts: [NPAIR, 128 c-part, CC, 128 d]
    wqt = np.ascontiguousarray(
        Wq.T.astype(bf).reshape(CC, 128, NPAIR, 128).transpose(2, 1, 0, 3)
    )
    wkt = np.ascontiguousarray(
        Wk.T.astype(bf).reshape(CC, 128, NPAIR, 128).transpose(2, 1, 0, 3)
    )
    # V weights in 4-pair (512 v-dim) group slabs: [NVG, 128 c-part, CC, 512]
    wvg = np.ascontiguousarray(
        Wv.T.astype(bf).reshape(CC, 128, NVG, 512).transpose(2, 1, 0, 3)
    )
    wpt = np.ascontiguousarray(W_proj.T.astype(bf))

    # RoPE tables in transposed/replicated layout:
    #   cosr[r, t] = cos[t, r % 32]
    #   sinB[r, t] = +sin[t, r%32] for (r%64)<32 else -sin[t, r%32]
    cosT = cos.T
    sinT = sin.T
    cosr = np.ascontiguousarray(np.tile(cosT, (4, 1)).astype(bf))
    sinB = np.ascontiguousarray(
        np.tile(np.concatenate([sinT, -sinT], axis=0), (2, 1)).astype(bf)
    )

    in_maps = []
    for c in range(NCORES):
        b, hf = divmod(c, 2)
        qs = hf * TQ
        # token order per core: own q half first, partner half second
        # (attention is permutation-invariant over k tokens as long as
        # KT / V / rope tables all use the same order)
        ordr = np.concatenate(
            [np.arange(qs, qs + TQ), np.arange((TQ + qs) % T, (TQ + qs) % T + TQ)]
        )
        xtb = np.ascontiguousarray(x[b].T.astype(bf)[:, ordr])
        in_maps.append(
            {
                "xt": xtb,
                "wqt": wqt,
                "wkt": wkt,
                "wvg": wvg,
                "wpt": wpt,
                "cosk": np.ascontiguousarray(cosr[:, ordr]),
                "sink": np.ascontiguousarray(sinB[:, ordr]),
            }
        )

    nc = _get_nc()
    trace = bool(int(os.environ.get("BASSK_TRACE", "0")))
    res = bass_utils.run_bass_kernel_spmd(
        nc, in_maps, core_ids=list(range(NCORES)), trace=trace
    )
    if trace:
        kernel.last_exec_time_ns = res.exec_time_ns
        kernel.last_profile = res

    out = np.empty((B, T, C), dtype=np.float32)
    for c in range(NCORES):
        b, hf = divmod(c, 2)
        qs = hf * TQ
        out[b, qs:qs + TQ, :] = res.results[c]["out"]
    return out


# revision 14
# speedup vs baseline: 1.3462x; 1.0053x over previous
"""Distributed Trainium2 Bass kernel for nn_Attention_62766652063769 (v6).

Reference computation (B=4, T=2048, C=1024, H=16, HD=64):
    qkv = x @ W_qkv^T ; split into q, k, v heads
    q, k <- RoPE(q), RoPE(k)   (interleaved-pair rotation)
    attn = softmax(q k^T / sqrt(HD))   (mask is all-ones -> no masking)
    out  = (attn @ v) @ W_proj^T

Sharding (tensor-parallel over heads, per the sharding hint): core
c = 2*b + hh owns batch b and head-half hh (8 of 16 heads), over the
FULL 2048 query tokens.  Q/K/V projections and attention are computed
only for the core's own heads (no redundant K/V work); the output
projection is row-sharded over the core's 512 att channels, producing
a bf16 PARTIAL result per core which the host sums per batch at
gather time (out[b] = partial[2b] + partial[2b+1]).

Device structure per core (4 local head pairs x 2 query-half
sub-sweeps of 1024 q tokens, identical inner loop each):
  - Merged score tiles: one PSUM tile [128, 1024] per (kt, q-chunk)
    holds BOTH heads side by side via tile_position (0,0)/(64,0) ->
    row-tiled concurrent matmuls, exp starts after 2 MMs.
  - ACT runs ONLY Exp (one table load).  Softmax denominator
    reciprocal is computed on a compact [128, 16] reshape (DRAM
    round-trip) so the slow DVE reciprocal touches 16 elems/lane.
  - OT (attn @ V) deferred 2 kt-tiles, A,A,B,B stationary reuse,
    65th V column accumulates the softmax denominator.
  - Filler grants (next Q/K/V projections + rope) are interleaved
    into the ACT-bound attention loop in PAIRS (grant PSUM tiles
    share the score ring; odd counts shift its recycle phase).
  - Proj phase: per 128-token tile, head/tail split so the last
    pair's late-normalized att never blocks the in-order PE queue;
    bf16 output halves DMA'd on sync+scalar queues.
"""

import os
import re
import sys
import types

if "/opt/trn_rl_repo" not in sys.path:
    sys.path.insert(0, "/opt/trn_rl_repo")

import ml_dtypes
import numpy as np

import bass_rust
import concourse.bass as bass
import concourse.mybir as mybir
from concourse import bass_utils
from concourse.tile import TileContext, ScopedClock

# ---------------------------------------------------------------------------
# Environment patches (same as v1/v2)
# ---------------------------------------------------------------------------

def _patched_drain_and_barrier(self, tick_clock, wait_clock):
    """The walrus build in this container encodes at most one sync-wait per
    instruction; Tile's tail drain carries one wait per live semaphore.
    Emit single-wait NOPs on SP instead, then an unguarded drain."""
    gc = tick_clock.global_clock
    ticks = [int(x) for x in re.findall(r"\d+", repr(gc))]
    for i, t in enumerate(ticks):
        if t <= 0:
            continue
        l = [0] * len(ticks)
        l[i] = t
        nop = self.nc.sync.nop(nofuse=True)
        wait_clock.add_sem_waits(nop.ins, ScopedClock({None: bass_rust.VectorClock(l)}))
    self.nc.sync.drain()
    self.nc.all_engine_barrier()
    assert self.sems is not None
    popped = self.nc._tile_sem_poison_stack.pop()
    assert popped is self._sem_poison
    self.nc.clear_and_free_semaphores(list(self.sems.allocated().values()))
    self.nc.all_engine_barrier()


TileContext._drain_and_barrier = _patched_drain_and_barrier


def _split_multi_waits(nc):
    """Move extra sync-waits onto single-wait NOPs inserted just before the
    owning instruction on the same (in-order) engine."""
    for func in nc.m.functions:
        for bb in func.blocks:
            insts = bb.instructions
            if not any(
                i.sync_info is not None
                and i.sync_info.on_wait
                and len(i.sync_info.on_wait) > 1
                for i in insts
            ):
                continue
            new = []
            for inst in insts:
                si = inst.sync_info
                if si is not None and si.on_wait and len(si.on_wait) > 1:
                    waits = list(si.on_wait)
                    for w in waits[:-1]:
                        nop = mybir.InstNoOp(
                            name=nc.get_next_instruction_name(),
                            engine=inst.engine,
                            bass_nofuse=True,
                            sync_info=mybir.SyncInfo(on_wait=[w], on_update=[]),
                        )
                        nc.register_instruction(nop)
                        new.append(nop)
                    inst.sync_info = mybir.SyncInfo(
                        on_wait=[waits[-1]], on_update=list(si.on_update)
                    )
                new.append(inst)
            bb.instructions = new


def _install_ntff_hook():
    """Recreate antenv.axon_hooks (absent in this image) so
    run_bass_kernel_spmd(trace=True) can profile through libaxon_pjrt."""
    if "antenv.axon_hooks" in sys.modules:
        return
    import contextlib
    import ctypes

    mod = types.ModuleType("antenv.axon_hooks")
    _state = {"hook": None}

    def set_axon_ntff_profile_hook(hook):
        _state["hook"] = hook

    def get_axon_ntff_profile_hook():
        return _state["hook"]

    def _ntff_profile_via_ctypes(so_path):
        lib = ctypes.CDLL(so_path)
        if not hasattr(lib, "axon_start_nrt_profile"):
            return None
        lib.axon_start_nrt_profile.argtypes = [
            ctypes.POINTER(ctypes.c_int64),
            ctypes.c_size_t,
        ]
        lib.axon_start_nrt_profile.restype = ctypes.c_int64
        lib.axon_stop_nrt_profile.argtypes = [ctypes.c_char_p]
        lib.axon_stop_nrt_profile.restype = ctypes.c_int64

        @contextlib.contextmanager
        def _hook(output_dir, device_ids):
            import jax

            jax.devices()
            if device_ids:
                ids = (ctypes.c_int64 * len(device_ids))(*device_ids)
                rc = lib.axon_start_nrt_profile(ids, len(device_ids))
            else:
                rc = lib.axon_start_nrt_profile(None, 0)
            if rc != 0:
                raise RuntimeError(f"axon_start_nrt_profile rc={rc}")
            try:
                yield
            finally:
                n = lib.axon_stop_nrt_profile(str(output_dir).encode())
                if n < 0:
                    raise RuntimeError(f"axon_stop_nrt_profile rc={n}")
                print(f"profile: {n} file(s) in {output_dir}", file=sys.stderr)

        return _hook

    mod.set_axon_ntff_profile_hook = set_axon_ntff_profile_hook
    mod.get_axon_ntff_profile_hook = get_axon_ntff_profile_hook
    try:
        set_axon_ntff_profile_hook(
            _ntff_profile_via_ctypes("/opt/axon/libaxon_pjrt.so")
        )
    except Exception:
        pass
    sys.modules["antenv.axon_hooks"] = mod
    try:
        import antenv

        antenv.axon_hooks = mod
    except ImportError:
        pass


_install_ntff_hook()

# ---------------------------------------------------------------------------
# Problem constants
# ---------------------------------------------------------------------------

B, T, C = 4, 2048, 1024
H, HD = 16, 64
NCORES = 8
TQ = T // 2          # q tokens per sub-sweep
NPL = 4              # local head pairs per core (8 heads)
NVG = 2              # V groups of 2 local pairs (4 heads, 256 v dims)
KT_TILES = T // 128  # 16
SCALE = 1.0 / np.sqrt(HD)

F32 = mybir.dt.float32
BF16 = mybir.dt.bfloat16

CC = C // 128  # 8 contraction chunks


# ---------------------------------------------------------------------------
# Device program
# ---------------------------------------------------------------------------

def _build_nc():
    nc = bass.Bass(trn_type="TRN2", target_bir_lowering=False, debug=False)

    xt = nc.declare_dram_parameter("xt", [C, T], BF16, isOutput=False)
    wqt = nc.declare_dram_parameter("wqt", [NPL, 128, CC, 128], BF16,
                                    isOutput=False)
    wkt = nc.declare_dram_parameter("wkt", [NPL, 128, CC, 128], BF16,
                                    isOutput=False)
    wvg = nc.declare_dram_parameter("wvg", [NVG, 128, CC, 256], BF16,
                                    isOutput=False)
    wpt = nc.declare_dram_parameter("wpt", [NPL * 128, C], BF16,
                                    isOutput=False)
    cosk = nc.declare_dram_parameter("cosk", [128, T], BF16, isOutput=False)
    sink = nc.declare_dram_parameter("sink", [128, T], BF16, isOutput=False)
    out_ext = nc.declare_dram_parameter("out", [T, C], BF16,
                                       isOutput=True)

    rs_dram = nc.dram_tensor("rs_scratch", [NPL, 2, 2, TQ], BF16)
    rc_dram = nc.dram_tensor("rc_scratch", [NPL, 2, 2, TQ], BF16)

    with TileContext(nc) as tc:
        with tc.tile_pool(name="persist", bufs=1) as persist:
            qt_sb = persist.tile([128, NPL, T], BF16, tag="qt")
            att_sb = persist.tile([128, NPL, T], BF16, tag="att")
            kt_sb = persist.tile([128, NPL, T], BF16, tag="kt")
            v_sb = persist.tile([128, KT_TILES, 8, 65], BF16, tag="v")
            ck = persist.tile([128, T], BF16, tag="ck")
            sk = persist.tile([128, T], BF16, tag="sk")

            with tc.tile_pool(name="xtpool", bufs=1) as xtpool, \
                 tc.tile_pool(name="pw", bufs=1) as pw:
                xt_sb = xtpool.tile([128, CC, T], BF16, tag="xt")
                xt_r = xt.rearrange("(cc p) t -> p cc t", p=128)
                nc.vector.memset(v_sb[:, :, :, 64:65], 1.0)

                wp_sb = pw.tile([128, NPL, C], BF16)

                _attention(nc, tc, xt_sb, qt_sb, kt_sb, v_sb, att_sb,
                           ck, sk, wqt, wkt, wvg, rs_dram, rc_dram,
                           wp_sb, wpt, xt_r, cosk, sink)

                _phase_proj(nc, tc, wp_sb, att_sb, out_ext)

    _split_multi_waits(nc)
    return nc


def _attention(nc, tc, xt_sb, qt_sb, kt_sb, v_sb, att_sb, ck, sk,
               wqt, wkt, wvg, rs_dram, rc_dram, wp_sb, wpt, xt_r, cosk,
               sink):
    with tc.tile_pool(name="ring", bufs=2, space="PSUM") as ring, \
         tc.tile_pool(name="otps", bufs=1, space="PSUM") as otps, \
         tc.tile_pool(name="wts", bufs=2) as wts, \
         tc.tile_pool(name="qkx", bufs=2) as qkx, \
         tc.tile_pool(name="ptp", bufs=6) as ptp, \
         tc.tile_pool(name="eps", bufs=1) as eps:

        wq_tiles = {}
        wk_tiles = {}
        wv_tiles = {}

        def load_wq(p, eng=None):
            t = wts.tile([128, CC, 128], BF16, tag="wq")
            (eng or nc.gpsimd).dma_start(out=t, in_=wqt[p])
            wq_tiles[p] = t

        def load_wk(p, eng=None):
            t = wts.tile([128, CC, 128], BF16, tag="wk")
            (eng or nc.gpsimd).dma_start(out=t, in_=wkt[p])
            wk_tiles[p] = t

        def load_wv(g, eng=None):
            t = wts.tile([128, CC, 256], BF16, tag="wv")
            (eng or nc.gpsimd).dma_start(out=t, in_=wvg[g])
            wv_tiles[g] = t

        def qk_grant(dst_sb, pair, wtile, col0, swap_eng=None):
            """One 512-token-column projection + rope grant -> dst_sb."""
            if swap_eng is None:
                swap_eng = nc.sync
            ps = ring.tile([128, TQ], F32, tag="ring", name=f"ps_{pair}_{col0}")
            for cc in range(CC):
                nc.tensor.matmul(
                    ps[:, 0:512],
                    lhsT=wtile[:, cc, :],
                    rhs=xt_sb[:, cc, col0:col0 + 512],
                    start=(cc == 0),
                    stop=(cc == CC - 1),
                )
            xb = qkx.tile([128, 512], BF16, tag="xb", name=f"xb_{pair}_{col0}")
            nc.vector.tensor_copy(xb, ps[:, 0:512])
            u = qkx.tile([128, 512], BF16, tag="u", name=f"u_{pair}_{col0}")
            v = qkx.tile([128, 512], BF16, tag="v", name=f"v_{pair}_{col0}")
            vs = qkx.tile([128, 512], BF16, tag="vs", name=f"vs_{pair}_{col0}")
            nc.vector.tensor_mul(u, xb, ck[:, col0:col0 + 512])
            nc.vector.tensor_mul(v, xb, sk[:, col0:col0 + 512])
            for blk in range(4):
                r = blk * 32
                s = (blk ^ 1) * 32
                swap_eng.dma_start(out=vs[r:r + 32, :], in_=v[s:s + 32, :])
            nc.gpsimd.tensor_add(dst_sb[:, pair, col0:col0 + 512], u, vs)

        def v_grant(g, tt):
            """V columns for group g (4 heads), token tile tt."""
            psv = ring.tile([128, 256], F32, tag="ring", name=f"psv_{g}_{tt}")
            for cc in range(CC):
                nc.tensor.matmul(
                    psv,
                    lhsT=xt_sb[:, cc, tt * 128:(tt + 1) * 128],
                    rhs=wv_tiles[g][:, cc, :],
                    start=(cc == 0),
                    stop=(cc == CC - 1),
                )
            nc.vector.tensor_copy(
                v_sb[:, tt, 4 * g:4 * g + 4, 0:64],
                psv.rearrange("p (h d) -> p h d", h=4),
            )

        # ------------------------------------------------------- startup
        load_wq(0, eng=nc.scalar)
        nc.sync.dma_start(out=xt_sb[:, :, 0:512], in_=xt_r[:, :, 0:512])
        load_wk(0, eng=nc.scalar)
        nc.scalar.dma_start(out=ck[:, 0:512], in_=cosk[:, 0:512])
        nc.scalar.dma_start(out=sk[:, 0:512], in_=sink[:, 0:512])
        nc.sync.dma_start(out=xt_sb[:, :, 512:1024], in_=xt_r[:, :, 512:1024])
        load_wv(0, eng=nc.gpsimd)
        load_wq(1)
        load_wk(1)
        nc.scalar.dma_start(out=ck[:, 512:1024], in_=cosk[:, 512:1024])
        nc.scalar.dma_start(out=sk[:, 512:1024], in_=sink[:, 512:1024])
        qk_grant(qt_sb, 0, wq_tiles[0], 0, swap_eng=nc.scalar)
        qk_grant(kt_sb, 0, wk_tiles[0], 0, swap_eng=nc.scalar)
        qk_grant(qt_sb, 0, wq_tiles[0], 512, swap_eng=nc.scalar)
        for tt0 in range(6):
            v_grant(0, tt0)
        nc.sync.dma_start(out=xt_sb[:, :, 1024:1536], in_=xt_r[:, :, 1024:1536])
        nc.sync.dma_start(out=xt_sb[:, :, 1536:2048], in_=xt_r[:, :, 1536:2048])
        nc.scalar.dma_start(out=ck[:, TQ:T], in_=cosk[:, TQ:T])
        nc.scalar.dma_start(out=sk[:, TQ:T], in_=sink[:, TQ:T])

        # -------------------------------------------- attention sub-sweeps
        # sub u = (pair p = u//2, q-half s = u%2); q cols qs..qs+1024
        for u in range(2 * NPL):
            p, s = divmod(u, 2)
            qs = s * TQ

            def mk_v(g, tt):
                return lambda: v_grant(g, tt)

            def mk_k(q, n):
                return lambda: qk_grant(kt_sb, q, wk_tiles[q], n * 512)

            def mk_q(q, n):
                return lambda: qk_grant(qt_sb, q, wq_tiles[q], n * 512)

            # Grants allocate PSUM from the same ring as the score
            # tiles; an ODD number of grant allocations in a kt slot
            # shifts the ring phase so the next S tile waits on a
            # same-kt exp (~1-2us stall).  Emit grants in PAIRS.
            fill_at = {}
            if u == 0:
                fill_at = {
                    0: [mk_v(0, 6), mk_v(0, 7)],
                    1: [mk_k(0, 1), mk_k(0, 2)],
                    3: [mk_v(0, 8), mk_v(0, 9)],
                    5: [mk_v(0, 10), mk_v(0, 11)],
                    7: [mk_k(0, 3), mk_v(0, 12)],
                    9: [mk_v(0, 13), mk_v(0, 14)],
                    11: [mk_v(0, 15), mk_q(0, 2)],
                    13: [mk_q(0, 3)],
                }
            elif u in (1, 3, 5):
                q = p + 1
                fill_at = {
                    0: [mk_q(q, 0), mk_q(q, 1)],
                    2: [mk_k(q, 0), mk_k(q, 1)],
                    4: [mk_k(q, 2), mk_k(q, 3)],
                }
                if u == 1:
                    fill_at[6] = [mk_v(1, 0), mk_v(1, 1)]
                    fill_at[8] = [mk_v(1, 2), mk_v(1, 3)]
                    fill_at[10] = [mk_v(1, 4)]
                elif u == 3:
                    fill_at[6] = [mk_v(1, 11), mk_v(1, 12)]
                    fill_at[8] = [mk_v(1, 13), mk_v(1, 14)]
                    fill_at[10] = [mk_v(1, 15)]
            elif u in (2, 4, 6):
                q = p
                fill_at = {0: [mk_q(q, 2), mk_q(q, 3)]}
                if u == 2:
                    fill_at[2] = [mk_v(1, 5), mk_v(1, 6)]
                    fill_at[4] = [mk_v(1, 7), mk_v(1, 8)]
                    fill_at[6] = [mk_v(1, 9), mk_v(1, 10)]

            if u == 1:
                load_wv(1)
            if u == 0:
                load_wq(2)
                load_wk(2)
            if u == 2:
                load_wq(3)
                load_wk(3)
            if u == 6:
                wp_r = wpt.rearrange("(cc p) e -> p cc e", p=128)
                nc.sync.dma_start(out=wp_sb[:, 0:2, :], in_=wp_r[:, 0:2, :])
                nc.sync.dma_start(out=wp_sb[:, 2:4, :], in_=wp_r[:, 2:4, :])

            psAB = otps.tile([128, 2, TQ], F32, tag="ot", name=f"psAB_{u}")

            def ot_mm_A(kt, pt0, pt1):
                nc.tensor.matmul(
                    psAB[0:65, 0, 0:512],
                    lhsT=v_sb[:, kt, 2 * p, :],
                    rhs=pt0[:, 0:512],
                    start=(kt == 0),
                    stop=(kt == KT_TILES - 1),
                )
                nc.tensor.matmul(
                    psAB[0:65, 0, 512:1024],
                    lhsT=v_sb[:, kt, 2 * p, :],
                    rhs=pt1[:, 0:512],
                    start=(kt == 0),
                    stop=(kt == KT_TILES - 1),
                )

            def ot_mm_B(kt, pt0, pt1):
                nc.tensor.matmul(
                    psAB[0:65, 1, 0:512],
                    lhsT=v_sb[:, kt, 2 * p + 1, :],
                    rhs=pt0[:, 512:1024],
                    start=(kt == 0),
                    stop=(kt == KT_TILES - 1),
                )
                nc.tensor.matmul(
                    psAB[0:65, 1, 512:1024],
                    lhsT=v_sb[:, kt, 2 * p + 1, :],
                    rhs=pt1[:, 512:1024],
                    start=(kt == 0),
                    stop=(kt == KT_TILES - 1),
                )

            pends = []
            for kt in range(KT_TILES):
                st0 = ring.tile([128, TQ], F32, tag="ring",
                                name=f"st0_{u}_{kt}")
                st1 = ring.tile([128, TQ], F32, tag="ring",
                                name=f"st1_{u}_{kt}")
                nc.tensor.matmul(
                    st0[:, 0:512],
                    lhsT=kt_sb[0:64, p, kt * 128:(kt + 1) * 128],
                    rhs=qt_sb[0:64, p, qs:qs + 512],
                    start=True, stop=True, tile_position=(0, 0),
                )
                nc.tensor.matmul(
                    st0[:, 512:1024],
                    lhsT=kt_sb[64:128, p, kt * 128:(kt + 1) * 128],
                    rhs=qt_sb[64:128, p, qs:qs + 512],
                    start=True, stop=True, tile_position=(64, 0),
                )
                pt0 = ptp.tile([128, TQ], BF16, tag="pt",
                               name=f"pt0_{u}_{kt}")
                nc.scalar.activation(
                    out=pt0, in_=st0,
                    func=mybir.ActivationFunctionType.Exp, scale=SCALE,
                )
                if len(pends) == 2:
                    ot_mm_A(*pends[0])
                nc.tensor.matmul(
                    st1[:, 0:512],
                    lhsT=kt_sb[0:64, p, kt * 128:(kt + 1) * 128],
                    rhs=qt_sb[0:64, p, qs + 512:qs + 1024],
                    start=True, stop=True, tile_position=(0, 0),
                )
                nc.tensor.matmul(
                    st1[:, 512:1024],
                    lhsT=kt_sb[64:128, p, kt * 128:(kt + 1) * 128],
                    rhs=qt_sb[64:128, p, qs + 512:qs + 1024],
                    start=True, stop=True, tile_position=(64, 0),
                )
                pt1 = ptp.tile([128, TQ], BF16, tag="pt",
                               name=f"pt1_{u}_{kt}")
                nc.scalar.activation(
                    out=pt1, in_=st1,
                    func=mybir.ActivationFunctionType.Exp, scale=SCALE,
                )
                if len(pends) == 2:
                    ot_mm_B(*pends.pop(0))
                pends.append((kt, pt0, pt1))
                for f in fill_at.get(kt, ()):
                    f()
            for e in pends:
                ot_mm_A(*e)
                ot_mm_B(*e)

            # --------------------------------------------- sub epilogue
            attU = eps.tile([65, 2, TQ], BF16, tag="attU", bufs=1,
                            name=f"attU_{u}")
            nc.vector.tensor_copy(attU, psAB[0:65, :, :])
            nc.sync.dma_start(out=rs_dram[p, s], in_=attU[64:65, :, :])
            den128 = eps.tile([128, 16], BF16, tag="den128", bufs=1,
                              name=f"den128_{u}")
            nc.sync.dma_start(out=den128, in_=rs_dram[p, s])
            denf = eps.tile([128, 16], F32, tag="denf", bufs=1,
                            name=f"denf_{u}")
            nc.vector.tensor_copy(denf, den128)
            recf = eps.tile([128, 16], F32, tag="recf", bufs=1,
                            name=f"recf_{u}")
            nc.vector.reciprocal(recf, denf)
            rec128 = eps.tile([128, 16], BF16, tag="rec128", bufs=1,
                              name=f"rec128_{u}")
            nc.vector.tensor_copy(rec128, recf)
            nc.sync.dma_start(out=rc_dram[p, s], in_=rec128)
            rbc = eps.tile([64, 2, TQ], BF16, tag="rbc", bufs=1,
                           name=f"rbc_{u}")
            nc.sync.dma_start(
                out=rbc[:, 0, :],
                in_=rc_dram[p, s, 0:1, :].broadcast_to([64, TQ]))
            nc.sync.dma_start(
                out=rbc[:, 1, :],
                in_=rc_dram[p, s, 1:2, :].broadcast_to([64, TQ]))
            attB = eps.tile([64, TQ], BF16, tag="attB", bufs=1,
                            name=f"attB_{u}")
            nc.vector.tensor_mul(att_sb[0:64, p, qs:qs + TQ],
                                 attU[0:64, 0, :], rbc[:, 0, :])
            nc.vector.tensor_mul(attB, attU[0:64, 1, :], rbc[:, 1, :])
            nc.sync.dma_start(out=att_sb[64:128, p, qs:qs + TQ], in_=attB)


def _phase_proj(nc, tc, wp_sb, att_sb, out_ext):
    """out_partial = att^T @ WpT-rows (my 512 channels), per 128-token
    tile over the full 2048 tokens."""
    with tc.tile_pool(name="pph", bufs=2) as pph, \
         tc.tile_pool(name="pps", bufs=4, space="PSUM") as pps:
        NTT = T // 128

        def head_mm(ps, tt):
            for lp in range(NPL - 1):
                for nch in range(2):
                    nc.tensor.matmul(
                        ps[:, nch * 512:(nch + 1) * 512],
                        lhsT=att_sb[:, lp, tt * 128:(tt + 1) * 128],
                        rhs=wp_sb[:, lp, nch * 512:(nch + 1) * 512],
                        start=(lp == 0),
                        stop=False,
                    )

        def tail_mm(ps, tt):
            for nch in range(2):
                nc.tensor.matmul(
                    ps[:, nch * 512:(nch + 1) * 512],
                    lhsT=att_sb[:, NPL - 1, tt * 128:(tt + 1) * 128],
                    rhs=wp_sb[:, NPL - 1, nch * 512:(nch + 1) * 512],
                    start=False,
                    stop=True,
                )
            o = pph.tile([128, C], BF16, tag="o", name=f"o_{tt}")
            if tt % 2 == 0:
                nc.vector.tensor_copy(o, ps)
            else:
                nc.scalar.activation(
                    out=o, in_=ps, func=mybir.ActivationFunctionType.Copy
                )
            r = tt * 128
            nc.sync.dma_start(out=out_ext[r:r + 128, 0:512], in_=o[:, 0:512])
            nc.scalar.dma_start(out=out_ext[r:r + 128, 512:1024],
                                in_=o[:, 512:1024])

        pend = []
        for tt in range(NTT):
            ps = pps.tile([128, C], F32, tag="ps", name=f"ps_{tt}")
            head_mm(ps, tt)
            pend.append((ps, tt))
            if len(pend) > 2:
                tail_mm(*pend.pop(0))
        for e in pend:
            tail_mm(*e)


_NC_CACHE = None


def _get_nc():
    global _NC_CACHE
    if _NC_CACHE is None:
        _NC_CACHE = _build_nc()
    return _NC_CACHE


# ---------------------------------------------------------------------------
# Host wrapper
# ---------------------------------------------------------------------------

def kernel(x, W_qkv, W_proj, cos, sin, mask):
    bf = ml_dtypes.bfloat16
    x = np.asarray(x, dtype=np.float32)
    W_qkv = np.asarray(W_qkv, dtype=np.float32)
    W_proj = np.asarray(W_proj, dtype=np.float32)
    cos = np.asarray(cos, dtype=np.float32)
    sin = np.asarray(sin, dtype=np.float32)

    # Permute q/k head dims: interleaved (x1,x2 pairs) -> halves [x1; x2].
    perm = np.concatenate([np.arange(0, HD, 2), np.arange(1, HD, 2)])
    Wq = W_qkv[0:C].reshape(H, HD, C)[:, perm, :].reshape(C, C)
    Wk = W_qkv[C:2 * C].reshape(H, HD, C)[:, perm, :].reshape(C, C)
    Wv = W_qkv[2 * C:3 * C]

    # full-H tiled layouts: [8 pairs, 128 c-part, CC, 128 d]
    wqt_full = np.ascontiguousarray(
        Wq.T.astype(bf).reshape(CC, 128, 8, 128).transpose(2, 1, 0, 3)
    )
    wkt_full = np.ascontiguousarray(
        Wk.T.astype(bf).reshape(CC, 128, 8, 128).transpose(2, 1, 0, 3)
    )
    # V weights in 4-head group slabs: [4 groups, 128 c-part, CC, 256]
    wvg_full = np.ascontiguousarray(
        Wv.T.astype(bf).reshape(CC, 128, 4, 256).transpose(2, 1, 0, 3)
    )
    wpt_full = W_proj.T.astype(bf)   # [1024 rows (channels), 1024]

    cosT = cos.T
    sinT = sin.T
    cosr = np.ascontiguousarray(np.tile(cosT, (4, 1)).astype(bf))
    sinB = np.ascontiguousarray(
        np.tile(np.concatenate([sinT, -sinT], axis=0), (2, 1)).astype(bf)
    )

    in_maps = []
    for c in range(NCORES):
        b, hh = divmod(c, 2)
        xtb = np.ascontiguousarray(x[b].T.astype(bf))
        in_maps.append(
            {
                "xt": xtb,
                "wqt": np.ascontiguousarray(wqt_full[4 * hh:4 * hh + 4]),
                "wkt": np.ascontiguousarray(wkt_full[4 * hh:4 * hh + 4]),
                "wvg": np.ascontiguousarray(wvg_full[2 * hh:2 * hh + 2]),
                "wpt": np.ascontiguousarray(
                    wpt_full[512 * hh:512 * hh + 512]),
                "cosk": cosr,
                "sink": sinB,
            }
        )

    nc = _get_nc()
    trace = bool(int(os.environ.get("BASSK_TRACE", "0")))
    res = bass_utils.run_bass_kernel_spmd(
        nc, in_maps, core_ids=list(range(NCORES)), trace=trace
    )
    if trace:
        kernel.last_exec_time_ns = res.exec_time_ns
        kernel.last_profile = res

    out = np.empty((B, T, C), dtype=np.float32)
    for b in range(B):
        out[b] = (res.results[2 * b]["out"].astype(np.float32)
                  + res.results[2 * b + 1]["out"].astype(np.float32))
    return out
